# revision 35
# baseline (speedup 1.0000x reference)
"""GAT node-classification kernel for Trainium2 (8 NeuronCores, SPMD).

Strategy (dst-node graph partitioning per the sharding hint):
  - Only destination nodes appearing in `ids` affect the output; edges into
    other nodes are dead code and are dropped.
  - Destination nodes are sorted by in-degree and packed 128 per tile in
    rank order, dealt so all 8 cores share the same per-tile degree
    profile.  Tiles are batched into pipeline groups whose edge-slot width
    is the group's max degree (+1 pseudo-neighbor column carrying the
    node's own features for a_dst) — the attention/message work scales
    with the group width instead of the global max degree.
  - Per-edge attention logits are 7-term feature contractions on the
    Vector/GpSimd engines in fp16 (2-byte DVE fast modes); reductions are
    pairwise trees of tensor_tensor adds whose last levels accumulate in
    fp32.  A per-(tile,head) max is subtracted before exp (exact softmax
    shift) so fp16 exp inputs stay in range.  Weighted message sums stay
    in the rank-7 feature basis (sum(alpha*(x@W)) == (sum(alpha*x))@W).
  - Per tile the normalized sums (plus an appended ones column carrying
    the GAT bias and folded classifier bias) are transposed once on the
    PE (3 tiles per pass, quadrant-aligned) and hit a single fp16 matmul
    with rhs = [Wb;gb | Wb@linWp' | -rowsum/HC], yielding the GAT output
    o, the classifier projection q, and -mean(o) in one pass.  LayerNorm's
    affine transform is folded into the classifier weights; 1/std is
    applied after the matmul; rstd = exp(-0.5*ln(var)) so every
    activation shares one hardware function table.
  - All device inputs are packed host-side so each SBUF partition's data
    is contiguous in DRAM (one descriptor set per transfer); input DMAs
    are issued up-front on the in-order Sync queue; As/Ad are computed
    redundantly in every partition to avoid a DRAM broadcast roundtrip.

The host does no floating-point arithmetic on tensor values: it only
filters/sorts/permutes (sharding layout, gathers, block-diagonal placement
of W) and builds 0/1, eye, ones and 0/-60000 masks; all float math and all
dtype conversion runs on the NeuronCores.
"""
import os
import sys

sys.path.insert(0, "/opt/trn_rl_repo")

import numpy as np

import concourse.bass as bass
import concourse.bacc as bacc
import concourse.mybir as mybir
import concourse.tile as tile
from concourse import bass_utils
import concourse.bacc as _bacc_mod
import concourse.hw_specs as _hw_specs

_PIN_SET = "natural_log_exp_and_others"
_orig_get_tables = _hw_specs.get_activation_tables


def _pinned_tables(arch):
    """Route every activation to one table set (exp/ln/square/copy all
    coexist there) so the kernel pays a single ACT_TABLE_LOAD."""
    tabs = _orig_get_tables(arch)
    if _PIN_SET in tabs:
        tabs = {k: (v if k == _PIN_SET else set()) for k, v in tabs.items()}
    return tabs


_bacc_mod.get_activation_tables = _pinned_tables

N = 100000
FIN = 7
H = 4
C = 32
HC = H * C  # 128
CLS = 7
NEG = 0.2
NCORES = 8
F32 = mybir.dt.float32
F32R = mybir.dt.float32r
F16 = mybir.dt.float16

# const-pack column offsets (cursor-built)
_cur = 0


def _adv(w):
    global _cur
    o = _cur
    _cur += w
    return o


C_ID = _adv(128)     # identity [128,128]
C_WB = _adv(136)     # [Wb;gb] quadrant-replicated (128) | Wbc (7) | -rowsum/HC
C_WT = _adv(128)     # WT4G [128,125] (pad 3)
C_LIN = _adv(14)     # linW | (linWp written by device)
C_LNB = _adv(1)      # lnb column
C_ONE = _adv(1)      # ones column (adjacent: lhsT [128,2])
C_LNW = _adv(1)      # lnw column
C_EPS = _adv(1)      # 1e-5 column
C_E16 = _adv(1)      # 1e-16 column
C_SEL = _adv(14)     # rows 0:2 = keep-mask for [lbp | colsum] assembly
C_LB2 = _adv(14)     # row 0 cols 0:7 = lin_b, else 0
C_O2 = _adv(128)     # rows 0:2 = ones
NC = _cur + (-_cur % 8)


# ---------------------------------------------------------------- host prep
def _preprocess(x, edge_index, ids):
    src = np.asarray(edge_index[0], dtype=np.int64)
    dst = np.asarray(edge_index[1], dtype=np.int64)
    ids = np.asarray(ids, dtype=np.int64)
    x = np.asarray(x, np.float32)

    uids, inv = np.unique(ids, return_inverse=True)
    U = uids.shape[0]
    mark = np.full(N, -1, np.int64)
    mark[uids] = np.arange(U)

    dstc = mark[dst]
    keep = dstc >= 0
    es = src[keep]
    ed = dstc[keep]
    order = np.argsort(ed, kind="stable")
    es = es[order]
    ed = ed[order]
    cnt = np.bincount(ed, minlength=U).astype(np.int64)
    starts = np.zeros(U + 1, np.int64)
    np.cumsum(cnt, out=starts[1:])

    T_need = -(-U // 128)
    T_pc = -(-T_need // NCORES)
    T_pc += T_pc % 2
    T_tot = T_pc * NCORES
    Upad = T_tot * 128
    pads = Upad - U

    # degree-ascending rank: pad slots first, then nodes sorted by degree.
    # rank r -> tile t=r//1024, core c=(r//128)%8, slot s=r%128, so every
    # core sees the same per-tile degree profile.
    perm = np.argsort(cnt, kind="stable")
    rank_cnt = np.zeros(Upad, np.int64)
    rank_cnt[pads:] = cnt[perm]
    rank_uid = np.zeros(Upad, np.int64)
    rank_uid[pads:] = uids[perm]
    rank_uidx = np.full(Upad, -1, np.int64)
    rank_uidx[pads:] = perm
    rank_starts = np.zeros(Upad, np.int64)
    rank_starts[pads:] = starts[perm]

    # per-tile max degree -> groups of tiles with shared width
    tile_max = np.maximum(rank_cnt.reshape(T_pc, 8 * 128).max(1), 1)
    G_LIST = [4] * (T_pc // 4) + ([T_pc % 4] if T_pc % 4 else [])
    D_LIST = []
    t0 = 0
    for g in G_LIST:
        D_LIST.append(int(tile_max[t0 : t0 + g].max()))
        t0 += g

    blocks = []
    t0 = 0
    for G, D_PAD in zip(G_LIST, D_LIST):
        DE = D_PAD + 1
        COLT = 8 * DE
        r0, r1 = t0 * 1024, (t0 + G) * 1024
        nrow = r1 - r0
        sp = np.zeros((nrow, DE), np.int64)
        c0 = rank_cnt[r0:r1]
        st = rank_starts[r0:r1]
        # scatter edges: rows sorted by rank; edge k of row i at col k
        rows = np.repeat(np.arange(nrow), c0)
        cols = np.arange(rows.shape[0]) - np.repeat(
            np.cumsum(c0) - c0, c0)
        eidx = (np.repeat(st, c0) + cols)
        sp[rows, cols] = es[eidx]
        sp[:, D_PAD] = rank_uid[r0:r1]
        xgB = np.zeros((nrow, COLT), np.float32)
        xgB[:, : FIN * DE] = (
            x[sp.reshape(-1)].reshape(nrow, DE, FIN)
            .transpose(0, 2, 1).reshape(nrow, FIN * DE))
        j = np.arange(DE)[None, :]
        xgB[:, FIN * DE :] = np.where(
            j < c0[:, None], 0.0, -60000.0).astype(np.float32)
        blocks.append(np.ascontiguousarray(
            xgB.reshape(G, NCORES, 128, COLT)
            .transpose(1, 2, 0, 3)
            .reshape(NCORES, 128, G * COLT)))
        t0 += G
    xg2 = np.concatenate(blocks, axis=2)

    # output row of node-uidx u: device rows ordered (core, tile, slot)
    r = np.arange(pads, Upad)
    t, c, s = r // 1024, (r // 128) % 8, r % 128
    row_of_u = np.empty(U, np.int64)
    row_of_u[perm] = (c * T_pc + t) * 128 + s
    core_of_u = np.empty(U, np.int64)
    core_of_u[perm] = c

    return {"T_pc": T_pc, "G_LIST": tuple(G_LIST), "D_LIST": tuple(D_LIST),
            "xg2": xg2, "inv": inv, "row_of_u": row_of_u,
            "core_of_u": core_of_u}


def _const_pack(W, att_src, att_dst, gat_bias, ln_w, ln_b, lin_W, lin_b):
    W = np.ascontiguousarray(W, np.float32).reshape(FIN, HC)
    attS = np.ascontiguousarray(att_src, np.float32).reshape(HC)
    attD = np.ascontiguousarray(att_dst, np.float32).reshape(HC)
    gb = np.ascontiguousarray(gat_bias, np.float32).reshape(HC)
    cp = np.zeros((128, NC), np.float32)
    cp[:, C_ID : C_ID + 128] = np.eye(128, dtype=np.float32)
    # WbFull rows 32q+r: r<28 -> Wb row r ((h,f)=divmod(r,7)); r==28 -> gb
    wb = np.zeros((32, 128), np.float32)
    for r in range(28):
        h, f = divmod(r, FIN)
        wb[r, h * C : (h + 1) * C] = W[f, h * C : (h + 1) * C]
    wb[28, :] = gb
    for q in range(4):
        cp[32 * q : 32 * (q + 1), C_WB : C_WB + 128] = wb
    # WT4G col 32q+m: m<28 -> Wb row m transposed; m==28 -> gb
    wt = np.zeros((128, 32), np.float32)
    wt[:, :29] = wb[:29].T
    cp[:, C_WT : C_WT + 125] = np.tile(wt, (1, 4))[:, :125]
    cp[:, C_LIN : C_LIN + CLS] = np.ascontiguousarray(
        lin_W, np.float32).reshape(HC, CLS)
    cp[:, C_LNB] = np.ascontiguousarray(ln_b, np.float32).reshape(HC)
    cp[:, C_ONE] = 1.0
    cp[:, C_LNW] = np.ascontiguousarray(ln_w, np.float32).reshape(HC)
    cp[:, C_EPS] = 1e-5
    cp[:, C_E16] = 1e-16
    cp[0, C_SEL : C_SEL + CLS] = 1.0
    cp[1, C_SEL + CLS : C_SEL + 14] = 1.0
    cp[0, C_LB2 : C_LB2 + CLS] = np.ascontiguousarray(
        lin_b, np.float32).reshape(CLS)
    cp[0:2, C_O2 : C_O2 + 128] = 1.0
    cp2 = np.empty((128, 1152), np.float32)
    cp2[:, 0:896] = W.reshape(-1)[None, :]
    cp2[:, 896:1024] = attS[None, :]
    cp2[:, 1024:1152] = attD[None, :]
    return np.ascontiguousarray(np.concatenate([cp, cp2], axis=1))


def _ap(base, off_elems, dims):
    """AP with explicit free dims; dims = [[step, count], ...]."""
    return bass.AP(base.tensor, base.offset + off_elems,
                   [list(base.ap[0])] + dims)


# ---------------------------------------------------------------- program
def _build(T_pc, G_LIST, D_LIST):
    nc = bacc.Bacc("TRN2", target_bir_lowering=False, debug=False,
                   num_devices=NCORES)
    NG = len(G_LIST)
    G_MAX = max(G_LIST)
    D_MAX = max(D_LIST)
    DE_MAX = D_MAX + 1
    JF_MAX = FIN * DE_MAX
    COLT_L = [8 * (d + 1) for d in D_LIST]
    TOTC = sum(g * c for g, c in zip(G_LIST, COLT_L))
    # per-level max widths of the generic j-tree (for tile allocation)
    halves_max = []
    n = D_MAX
    while n > 1:
        halves_max.append(n // 2)
        n //= 2

    d_xg = nc.dram_tensor("xg2", [128, TOTC], F32, kind="ExternalInput")
    d_cp = nc.dram_tensor("cpack", [128, NC + 1152], F32,
                          kind="ExternalInput")
    d_out = nc.dram_tensor("probs", [128, T_pc * CLS], F32,
                           kind="ExternalOutput")

    AX = mybir.AxisListType.X
    OP = mybir.AluOpType
    ACT = mybir.ActivationFunctionType

    with tile.TileContext(nc) as tc:
        with (
            tc.tile_pool(name="const", bufs=1) as cp,
            tc.tile_pool(name="work", bufs=2) as wp,
            tc.tile_pool(name="pp_p", bufs=1, space="PSUM") as pp_p,
            tc.tile_pool(name="pp_t", bufs=2, space="PSUM") as pp_t,
            tc.tile_pool(name="pp_o", bufs=2, space="PSUM") as pp_o,
        ):
            # ---- prologue: packed const DMAs, then on-device weight prep
            CP = cp.tile([128, NC + 1152], F32, tag="CP")
            nc.sync.dma_start(out=CP[:], in_=d_cp[:, :])
            CP2 = CP[:, NC : NC + 1152]
            ident = CP[:, C_ID : C_ID + 128]

            # main-input DMAs issued up-front (in-order Sync queue)
            goff0 = [0]
            for g in range(NG):
                goff0.append(goff0[-1] + G_LIST[g] * COLT_L[g])
            xgs = {}
            for g in range(NG):
                w = G_LIST[g] * COLT_L[g]
                xgt = wp.tile([128, G_MAX * 8 * DE_MAX], F32, tag="xg")
                nc.sync.dma_start(out=xgt[:, 0:w],
                                  in_=d_xg[:, goff0[g] : goff0[g] + w])
                xgs[g] = xgt

            # As/Ad computed redundantly in every partition:
            # a78[p,(w,f,h)] = sum_c W[f,hc]*att_w[hc]
            t78 = cp.tile([128, 1792], F32, tag="t78")
            nc.gpsimd.tensor_tensor(
                out=_ap(t78[:], 0, [[896, 2], [128, FIN], [1, 128]]),
                in0=_ap(CP2, 0, [[0, 2], [128, FIN], [1, 128]]),
                in1=_ap(CP2, 896, [[128, 2], [0, FIN], [1, 128]]),
                op=OP.mult)
            a78 = cp.tile([128, 56], F32, tag="a78")
            nc.vector.tensor_reduce(
                out=_ap(a78[:], 0, [[4, 14], [1, 4]]),
                in_=_ap(t78[:], 0, [[128, 14], [32, 4], [1, 32]]),
                axis=AX, op=OP.add)
            # expanded fp16 (h,f,j) table of As (a_dst handled separately;
            # the pseudo-neighbor column j=D_PAD is masked anyway)
            AsE = cp.tile([128, H * JF_MAX], F16, tag="AsE")
            nc.scalar.activation(
                out=_ap(AsE[:], 0, [[JF_MAX, H], [DE_MAX, FIN], [1, DE_MAX]]),
                in_=_ap(a78[:], 0, [[1, H], [4, FIN], [0, DE_MAX]]),
                func=ACT.Copy)

            # linWp = lnw * linW (cols 7:14 of lin2 region, inside CP)
            nc.vector.tensor_scalar(
                out=CP[:, C_LIN + CLS : C_LIN + 2 * CLS],
                in0=CP[:, C_LIN : C_LIN + CLS],
                scalar1=CP[:, C_LNW : C_LNW + 1], scalar2=None, op0=OP.mult)
            # Wbc[32q+m, k] = (Wb@linWp')[m,k] (m<28) / (gb@linWp')[k] (m=28)
            ps_w = pp_p.tile([125, CLS], F32, tag="psw")
            nc.tensor.matmul(
                out=ps_w[:], lhsT=CP[:, C_WT : C_WT + 125],
                rhs=CP[:, C_LIN + CLS : C_LIN + 2 * CLS],
                start=True, stop=True)
            WbF = cp.tile([128, 136], F16, tag="WbF")
            nc.scalar.activation(out=WbF[:, 0:128],
                                 in_=CP[:, C_WB : C_WB + 128], func=ACT.Copy)
            nc.scalar.activation(out=WbF[0:125, 128:135], in_=ps_w[:],
                                 func=ACT.Copy)
            id16 = cp.tile([128, 128], F16, tag="id16")
            nc.scalar.activation(out=id16[:], in_=ident, func=ACT.Copy)
            # wsum col: -(row sum of [Wb;gb]) / HC  -> matmul emits -mean(o)
            ws = cp.tile([128, 1], F32, tag="ws")
            nc.vector.tensor_reduce(
                out=ws[:], in_=CP[:, C_WB : C_WB + 128], axis=AX, op=OP.add)
            nc.scalar.activation(out=WbF[:, 135:136], in_=ws[:],
                                 func=ACT.Copy, scale=-1.0 / HC)
            # [lbp | colsum] broadcast rows
            ps_a = pp_p.tile([2, 14], F32, tag="psa")
            nc.tensor.matmul(
                out=ps_a[:], lhsT=CP[:, C_LNB : C_LNB + 2],
                rhs=CP[:, C_LIN : C_LIN + 14],
                start=True, stop=True)
            z2a = cp.tile([2, 14], F32, tag="z2a")
            nc.vector.tensor_tensor(out=z2a[:], in0=ps_a[:],
                                    in1=CP[0:2, C_SEL : C_SEL + 14],
                                    op=OP.mult)
            z2 = cp.tile([2, 14], F32, tag="z2")
            nc.vector.tensor_tensor(out=z2[:], in0=z2a[:],
                                    in1=CP[0:2, C_LB2 : C_LB2 + 14],
                                    op=OP.add)
            ps_b = pp_p.tile([128, 14], F32, tag="psb")
            nc.tensor.matmul(
                out=ps_b[:], lhsT=CP[0:2, C_O2 : C_O2 + 128],
                rhs=z2[:], start=True, stop=True)
            LC = cp.tile([128, 14], F32, tag="LC")
            nc.scalar.copy(out=LC[:], in_=ps_b[:])

            # ---- main loop: NG groups, heaviest width first (the light
            # group's shorter serial tail then ends the kernel)
            toff = [0]
            for g in range(NG):
                toff.append(toff[-1] + G_LIST[g])
            for g in range(NG):
                G, D_PAD = G_LIST[g], D_LIST[g]
                t0 = toff[g]
                DE = D_PAD + 1
                COLT = 8 * DE
                JF = FIN * DE
                JH = H * DE
                MJF = FIN * D_PAD
                O_MK = JF
                NB = -(-G // 3)
                TH = G * H
                THF = TH * FIN
                xg = xgs[g]
                # fp16 cast, one op per tile for pipelining
                xb = wp.tile([128, G_MAX * 8 * DE_MAX], F16, tag="xb")
                for t in range(G):
                    nc.scalar.activation(
                        out=xb[:, t * COLT : (t + 1) * COLT],
                        in_=xg[:, t * COLT : (t + 1) * COLT], func=ACT.Copy)

                # a_dst: ad[s,(t,h)] = sum_f xslot*Ad  (from fp32 xg)
                pd = wp.tile([128, G_MAX * H * FIN], F32, tag="pd")
                nc.vector.tensor_tensor(
                    out=_ap(pd[:], 0, [[H * FIN, G], [FIN, H], [1, FIN]]),
                    in0=_ap(xg[:], D_PAD, [[COLT, G], [0, H], [DE, FIN]]),
                    in1=_ap(a78[:], 28, [[0, G], [1, H], [4, FIN]]),
                    op=OP.mult)
                adt = wp.tile([128, G_MAX * H], F32, tag="adt")
                nc.vector.tensor_reduce(
                    out=_ap(adt[:], 0, [[H, G], [1, H]]),
                    in_=_ap(pd[:], 0, [[H * FIN, G], [FIN, H], [1, FIN]]),
                    axis=AX, op=OP.add)
                adb = wp.tile([128, G_MAX * H], F16, tag="adb")
                nc.scalar.activation(out=adb[:, 0 : G * H],
                                     in_=adt[:, 0 : G * H], func=ACT.Copy)
                # a_src products: pa[s,(t,h,f,j)] = xb * AsE  (fp16 2x)
                pa = wp.tile([128, G_MAX * H * JF_MAX], F16, tag="pa")
                for t in range(G):
                    eng = nc.gpsimd if (D_PAD <= 20 and t % 2 == 1) else nc.vector
                    eng.tensor_tensor(
                        out=_ap(pa[:], t * H * JF,
                                [[JF, H], [DE, FIN], [1, DE]]),
                        in0=_ap(xb[:], t * COLT,
                                [[0, H], [DE, FIN], [1, DE]]),
                        in1=_ap(AsE[:], 0,
                                [[JF_MAX, H], [DE_MAX, FIN], [1, DE]]),
                        op=OP.mult)
                # tree-reduce over f (7 = 3+3+1): sE[s,(t,h,j)]
                q3 = wp.tile([128, G_MAX * H * 3 * DE_MAX], F16, tag="q3")
                nc.vector.tensor_tensor(
                    out=_ap(q3[:], 0, [[3 * DE, TH], [DE, 3], [1, DE]]),
                    in0=_ap(pa[:], 0, [[JF, TH], [DE, 3], [1, DE]]),
                    in1=_ap(pa[:], 3 * DE, [[JF, TH], [DE, 3], [1, DE]]),
                    op=OP.add)
                r1 = wp.tile([128, G_MAX * H * DE_MAX], F16, tag="r1")
                nc.vector.tensor_tensor(
                    out=_ap(r1[:], 0, [[DE, TH], [1, DE]]),
                    in0=_ap(q3[:], 0, [[3 * DE, TH], [1, DE]]),
                    in1=_ap(q3[:], DE, [[3 * DE, TH], [1, DE]]),
                    op=OP.add)
                r2 = wp.tile([128, G_MAX * H * DE_MAX], F16, tag="r2")
                nc.vector.tensor_tensor(
                    out=_ap(r2[:], 0, [[DE, TH], [1, DE]]),
                    in0=_ap(r1[:], 0, [[DE, TH], [1, DE]]),
                    in1=_ap(q3[:], 2 * DE, [[3 * DE, TH], [1, DE]]),
                    op=OP.add)
                sE = wp.tile([128, G_MAX * H * DE_MAX], F16, tag="sE")
                nc.vector.tensor_tensor(
                    out=_ap(sE[:], 0, [[DE, TH], [1, DE]]),
                    in0=_ap(r2[:], 0, [[DE, TH], [1, DE]]),
                    in1=_ap(pa[:], 6 * DE, [[JF, TH], [1, DE]]),
                    op=OP.add)
                # + mask, + a_dst bcast, leaky (gpsimd), then exp
                sF = wp.tile([128, G_MAX * H * DE_MAX], F16, tag="sF")
                (nc.vector if D_PAD <= 20 else nc.gpsimd).tensor_tensor(
                    out=_ap(sF[:], 0, [[JH, G], [DE, H], [1, DE]]),
                    in0=_ap(sE[:], 0, [[JH, G], [DE, H], [1, DE]]),
                    in1=_ap(xb[:], O_MK, [[COLT, G], [0, H], [1, DE]]),
                    op=OP.add)
                sG = wp.tile([128, G_MAX * H * DE_MAX], F16, tag="sG")
                (nc.vector if D_PAD <= 20 else nc.gpsimd).tensor_tensor(
                    out=_ap(sG[:], 0, [[JH, G], [DE, H], [1, DE]]),
                    in0=_ap(sF[:], 0, [[JH, G], [DE, H], [1, DE]]),
                    in1=_ap(adb[:], 0, [[H, G], [1, H], [0, DE]]),
                    op=OP.add)
                ezl = wp.tile([128, G_MAX * H * DE_MAX], F16, tag="ezl")
                nc.vector.scalar_tensor_tensor(
                    out=ezl[:, 0 : TH * DE], in0=sG[:, 0 : TH * DE],
                    scalar=NEG, in1=sG[:, 0 : TH * DE],
                    op0=OP.mult, op1=OP.max)
                mx = wp.tile([128, G_MAX * H], F16, tag="mx")
                nc.vector.tensor_reduce(
                    out=_ap(mx[:], 0, [[H, G], [1, H]]),
                    in_=_ap(ezl[:], 0, [[JH, G], [DE, H], [1, DE]]),
                    axis=AX, op=OP.max)
                ezm = wp.tile([128, G_MAX * H * DE_MAX], F16, tag="ezm")
                nc.gpsimd.tensor_tensor(
                    out=_ap(ezm[:], 0, [[JH, G], [DE, H], [1, DE]]),
                    in0=_ap(ezl[:], 0, [[JH, G], [DE, H], [1, DE]]),
                    in1=_ap(mx[:], 0, [[H, G], [1, H], [0, DE]]),
                    op=OP.subtract)
                ez = wp.tile([128, G_MAX * H * DE_MAX], F16, tag="ez")
                nc.scalar.activation(out=ez[:, 0 : TH * DE],
                                     in_=ezm[:, 0 : TH * DE], func=ACT.Exp)

                # denominators (fp32)
                den = wp.tile([128, G_MAX * H], F32, tag="den")
                nc.vector.tensor_reduce(
                    out=_ap(den[:], 0, [[H, G], [1, H]]),
                    in_=_ap(ez[:], 0, [[JH, G], [DE, H], [1, DE]]),
                    axis=AX, op=OP.add)

                # messages: pm[s,(t,h,f,j)] = ez * xb  (fp16 2x, j<D_PAD)
                pm = wp.tile([128, G_MAX * H * FIN * D_MAX], F16, tag="pm")
                for t in range(G):
                    eng = nc.gpsimd if (D_PAD <= 20 and t % 2 == 0) else nc.vector
                    eng.tensor_tensor(
                        out=_ap(pm[:], t * H * MJF,
                                [[MJF, H], [D_PAD, FIN], [1, D_PAD]]),
                        in0=_ap(ez[:], t * JH,
                                [[DE, H], [0, FIN], [1, D_PAD]]),
                        in1=_ap(xb[:], t * COLT,
                                [[0, H], [DE, FIN], [1, D_PAD]]),
                        op=OP.mult)
                # generic pairwise tree-reduce over j; odd leftovers folded
                # in at the end; small levels accumulate in fp32
                cur, stride, ncur = pm, D_PAD, D_PAD
                leftovers = []
                lvl = 0
                while ncur > 1:
                    half = ncur // 2
                    if ncur % 2:
                        leftovers.append((cur, stride, ncur - 1))
                    dt = F16 if half >= 4 else F32
                    hm = halves_max[lvl] if lvl < len(halves_max) else half
                    nxt = wp.tile([128, G_MAX * H * FIN * max(hm, half)],
                                  dt, tag=f"jt{lvl}")
                    nc.vector.tensor_tensor(
                        out=_ap(nxt[:], 0, [[half, THF], [1, half]]),
                        in0=_ap(cur[:], 0, [[stride, THF], [1, half]]),
                        in1=_ap(cur[:], half, [[stride, THF], [1, half]]),
                        op=OP.add)
                    cur, stride, ncur = nxt, half, half
                    lvl += 1
                for li, (buf, st, off) in enumerate(leftovers):
                    nxt = wp.tile([128, G_MAX * H * FIN], F32, tag=f"jl{li}")
                    nc.vector.tensor_tensor(
                        out=_ap(nxt[:], 0, [[1, THF]]),
                        in0=_ap(cur[:], 0, [[stride, THF]]),
                        in1=_ap(buf[:], off, [[st, THF]]),
                        op=OP.add)
                    cur, stride = nxt, 1
                SD = cur

                # normalize: Sn = SD/(den+eps), ones col 28 per tile
                rd = wp.tile([128, G_MAX * H], F32, tag="rd")
                nc.scalar.activation(out=rd[:, 0 : G * H],
                                     in_=den[:, 0 : G * H], func=ACT.Copy,
                                     bias=1e-16)
                nc.vector.reciprocal(out=rd[:, 0 : G * H],
                                     in_=rd[:, 0 : G * H])
                Sn = wp.tile([128, G_MAX * 32], F16, tag="Sn")
                nc.gpsimd.memset(_ap(Sn[:], 28, [[32, G]]), 1.0)
                nc.vector.tensor_tensor(
                    out=_ap(Sn[:], 0, [[32, G], [FIN, H], [1, FIN]]),
                    in0=_ap(SD[:], 0, [[H * FIN, G], [FIN, H], [1, FIN]]),
                    in1=_ap(rd[:], 0, [[H, G], [1, H], [0, FIN]]),
                    op=OP.mult)

                # transpose 3 tiles per PE pass; quadrant-aligned SnT
                SnTs = []
                for b in range(NB):
                    w = min(96, G * 32 - b * 96)
                    ps_t = pp_t.tile([96, 128], F16, tag="pst")
                    nc.tensor.transpose(out=ps_t[0:w, :],
                                        in_=Sn[:, b * 96 : b * 96 + w],
                                        identity=id16)
                    SnT = wp.tile([96, 128], F16, tag=f"snt{b}")
                    nc.scalar.activation(out=SnT[0:w, :], in_=ps_t[0:w, :],
                                         func=ACT.Copy)
                    SnTs.append(SnT)

                # per tile: [o | q | -mu] = SnT.T @ WbFull, then LN + logits
                vs = wp.tile([128, G_MAX], F32, tag="vs")
                nm = wp.tile([128, G_MAX], F32, tag="nm")
                u_sb = wp.tile([128, G_MAX * CLS], F32, tag="u")
                lg = wp.tile([128, G_MAX * CLS], F32, tag="lg")
                for t in range(G):
                    b, tb = divmod(t, 3)
                    base = tb * 32
                    ps_o = pp_o.tile([128, 136], F32, tag="pso")
                    nc.tensor.matmul(
                        out=ps_o[:],
                        lhsT=SnTs[b][base : base + 29, :],
                        rhs=WbF[base : base + 29, :],
                        start=True, stop=True)
                    nc.vector.tensor_scalar(
                        out=nm[:, t : t + 1], in0=ps_o[:, 135:136],
                        scalar1=1.0, scalar2=None, op0=OP.mult)
                    sqt = wp.tile([128, HC], F32, tag="sqt")
                    nc.scalar.activation(
                        out=sqt[:], in_=ps_o[:, 0:HC], func=ACT.Square,
                        bias=nm[:, t : t + 1], accum_out=vs[:, t : t + 1])
                    nc.vector.scalar_tensor_tensor(
                        out=u_sb[:, t * CLS : (t + 1) * CLS],
                        in0=LC[:, CLS:14], scalar=ps_o[:, 135:136],
                        in1=ps_o[:, HC : HC + CLS],
                        op0=OP.mult, op1=OP.add)
                nc.scalar.activation(out=vs[:, 0:G], in_=vs[:, 0:G],
                                     func=ACT.Ln, scale=1.0 / HC,
                                     bias=CP[:, C_EPS : C_EPS + 1])
                nc.scalar.activation(out=vs[:, 0:G], in_=vs[:, 0:G],
                                     func=ACT.Exp, scale=-0.5)
                nc.vector.tensor_tensor(
                    out=_ap(lg[:], 0, [[CLS, G], [1, CLS]]),
                    in0=_ap(u_sb[:], 0, [[CLS, G], [1, CLS]]),
                    in1=_ap(vs[:], 0, [[1, G], [0, CLS]]), op=OP.mult)
                nc.vector.tensor_tensor(
                    out=_ap(lg[:], 0, [[CLS, G], [1, CLS]]),
                    in0=_ap(lg[:], 0, [[CLS, G], [1, CLS]]),
                    in1=_ap(LC[:], 0, [[0, G], [1, CLS]]), op=OP.add)
                nc.scalar.activation(out=lg[:, 0 : G * CLS],
                                     in_=lg[:, 0 : G * CLS], func=ACT.Exp)
                se = wp.tile([128, G_MAX], F32, tag="se")
                nc.vector.tensor_reduce(
                    out=se[:, 0:G],
                    in_=_ap(lg[:], 0, [[CLS, G], [1, CLS]]),
                    axis=AX, op=OP.add)
                nc.vector.reciprocal(out=se[:, 0:G], in_=se[:, 0:G])
                po = wp.tile([128, G_MAX * CLS], F32, tag="po")
                nc.gpsimd.tensor_tensor(
                    out=_ap(po[:], 0, [[CLS, G], [1, CLS]]),
                    in0=_ap(lg[:], 0, [[CLS, G], [1, CLS]]),
                    in1=_ap(se[:], 0, [[1, G], [0, CLS]]), op=OP.mult)
                nc.sync.dma_start(
                    out=d_out[:, t0 * CLS : (t0 + G) * CLS],
                    in_=po[:, 0 : G * CLS])

    nc.compile()
    return nc


_CACHE = {}


def _program(T_pc, G_LIST, D_LIST):
    key = (T_pc, G_LIST, D_LIST)
    if key not in _CACHE:
        _CACHE[key] = _build(T_pc, G_LIST, D_LIST)
    return _CACHE[key]


# ---------------------------------------------------------------- entry
def kernel(x, edge_weight, W, att_src, att_dst, gat_bias, ln_w, ln_b,
           lin_W, lin_b, edge_index, ids):
    prep = _preprocess(np.asarray(x), np.asarray(edge_index),
                       np.asarray(ids))
    T_pc = prep["T_pc"]
    nc = _program(T_pc, prep["G_LIST"], prep["D_LIST"])
    cpack = _const_pack(W, att_src, att_dst, gat_bias, ln_w, ln_b,
                        lin_W, lin_b)

    in_maps = [{"xg2": prep["xg2"][c], "cpack": cpack}
               for c in range(NCORES)]

    if os.environ.get("KERNEL_SIM"):
        from concourse.bass_interp import CoreSim

        outs = []
        ncores = int(os.environ.get("KERNEL_SIM_CORES", "1"))
        for c in range(ncores):
            sim = CoreSim(nc, require_finite=False, require_nnan=False)
            for k, v in in_maps[c].items():
                sim.tensor(k)[:] = v
            sim.simulate()
            outs.append(sim.tensor("probs").copy())
        arr = np.stack(outs + [np.zeros_like(outs[0])] * (NCORES - ncores))
    else:
        trace = bool(int(os.environ.get("KERNEL_TRACE", "0")))
        res = bass_utils.run_bass_kernel_spmd(
            nc, in_maps, core_ids=list(range(NCORES)), trace=trace)
        if trace and res.exec_time_ns is not None:
            print(f"HW exec time: {res.exec_time_ns} ns")
        arr = np.stack([res.results[c]["probs"] for c in range(NCORES)])

    full = (arr.reshape(NCORES, 128, T_pc, CLS)
            .transpose(0, 2, 1, 3)
            .reshape(NCORES * T_pc * 128, CLS))
    return np.ascontiguousarray(
        full[prep["row_of_u"]][prep["inv"]], np.float32)


# revision 37
# speedup vs baseline: 1.0160x; 1.0160x over previous
"""GAT node-classification kernel for Trainium2 (8 NeuronCores, SPMD).

Strategy (dst-node graph partitioning per the sharding hint):
  - Only destination nodes appearing in `ids` affect the output; edges into
    other nodes are dead code and are dropped.
  - Destination nodes are sorted by in-degree and packed 128 per tile in
    rank order, dealt so all 8 cores share the same per-tile degree
    profile.  Tiles are batched into pipeline groups whose edge-slot width
    is the group's max degree (+1 pseudo-neighbor column carrying the
    node's own features for a_dst) — the attention/message work scales
    with the group width instead of the global max degree.
  - Per-edge attention logits are 7-term feature contractions on the
    Vector/GpSimd engines in fp16 (2-byte DVE fast modes); reductions are
    pairwise trees of tensor_tensor adds whose last levels accumulate in
    fp32.  A per-(tile,head) max is subtracted before exp (exact softmax
    shift) so fp16 exp inputs stay in range.  Weighted message sums stay
    in the rank-7 feature basis (sum(alpha*(x@W)) == (sum(alpha*x))@W).
  - Per tile the normalized sums (plus an appended ones column carrying
    the GAT bias and folded classifier bias) are transposed once on the
    PE (3 tiles per pass, quadrant-aligned) and hit a single fp16 matmul
    with rhs = [Wb;gb | Wb@linWp' | -rowsum/HC], yielding the GAT output
    o, the classifier projection q, and -mean(o) in one pass.  LayerNorm's
    affine transform is folded into the classifier weights; 1/std is
    applied after the matmul; rstd = exp(-0.5*ln(var)) so every
    activation shares one hardware function table.
  - All device inputs are packed host-side so each SBUF partition's data
    is contiguous in DRAM (one descriptor set per transfer); input DMAs
    are issued up-front on the in-order Sync queue; As/Ad are computed
    redundantly in every partition to avoid a DRAM broadcast roundtrip.

The host does no floating-point arithmetic on tensor values: it only
filters/sorts/permutes (sharding layout, gathers, block-diagonal placement
of W) and builds 0/1, eye, ones and 0/-60000 masks; all float math and all
dtype conversion runs on the NeuronCores.
"""
import os
import sys

sys.path.insert(0, "/opt/trn_rl_repo")

import numpy as np

import concourse.bass as bass
import concourse.bacc as bacc
import concourse.mybir as mybir
import concourse.tile as tile
from concourse import bass_utils
import concourse.bacc as _bacc_mod
import concourse.hw_specs as _hw_specs

_PIN_SET = "natural_log_exp_and_others"
_orig_get_tables = _hw_specs.get_activation_tables


def _pinned_tables(arch):
    """Route every activation to one table set (exp/ln/square/copy all
    coexist there) so the kernel pays a single ACT_TABLE_LOAD."""
    tabs = _orig_get_tables(arch)
    if _PIN_SET in tabs:
        tabs = {k: (v if k == _PIN_SET else set()) for k, v in tabs.items()}
    return tabs


_bacc_mod.get_activation_tables = _pinned_tables

N = 100000
FIN = 7
H = 4
C = 32
HC = H * C  # 128
CLS = 7
NEG = 0.2
NCORES = 8
F32 = mybir.dt.float32
F32R = mybir.dt.float32r
F16 = mybir.dt.float16

# const-pack column offsets (cursor-built)
_cur = 0


def _adv(w):
    global _cur
    o = _cur
    _cur += w
    return o


C_ID = _adv(128)     # identity [128,128]
C_WB = _adv(136)     # [Wb;gb] quadrant-replicated (128) | Wbc (7) | -rowsum/HC
C_WT = _adv(128)     # WT4G [128,125] (pad 3)
C_LIN = _adv(14)     # linW | (linWp written by device)
C_LNB = _adv(1)      # lnb column
C_ONE = _adv(1)      # ones column (adjacent: lhsT [128,2])
C_LNW = _adv(1)      # lnw column
C_EPS = _adv(1)      # 1e-5 column
C_E16 = _adv(1)      # 1e-16 column
C_SEL = _adv(14)     # rows 0:2 = keep-mask for [lbp | colsum] assembly
C_LB2 = _adv(14)     # row 0 cols 0:7 = lin_b, else 0
C_O2 = _adv(128)     # rows 0:2 = ones
NC = _cur + (-_cur % 8)


# ---------------------------------------------------------------- host prep
def _preprocess(x, edge_index, ids):
    src = np.asarray(edge_index[0], dtype=np.int64)
    dst = np.asarray(edge_index[1], dtype=np.int64)
    ids = np.asarray(ids, dtype=np.int64)
    x = np.asarray(x, np.float32)

    uids, inv = np.unique(ids, return_inverse=True)
    U = uids.shape[0]
    mark = np.full(N, -1, np.int64)
    mark[uids] = np.arange(U)

    dstc = mark[dst]
    keep = dstc >= 0
    es = src[keep]
    ed = dstc[keep]
    order = np.argsort(ed, kind="stable")
    es = es[order]
    ed = ed[order]
    cnt = np.bincount(ed, minlength=U).astype(np.int64)
    starts = np.zeros(U + 1, np.int64)
    np.cumsum(cnt, out=starts[1:])

    T_need = -(-U // 128)
    T_pc = -(-T_need // NCORES)
    T_pc += T_pc % 2
    T_tot = T_pc * NCORES
    Upad = T_tot * 128
    pads = Upad - U

    # degree-ascending rank: pad slots first, then nodes sorted by degree.
    # rank r -> tile t=r//1024, core c=(r//128)%8, slot s=r%128, so every
    # core sees the same per-tile degree profile.
    perm = np.argsort(cnt, kind="stable")
    rank_cnt = np.zeros(Upad, np.int64)
    rank_cnt[pads:] = cnt[perm]
    rank_uid = np.zeros(Upad, np.int64)
    rank_uid[pads:] = uids[perm]
    rank_uidx = np.full(Upad, -1, np.int64)
    rank_uidx[pads:] = perm
    rank_starts = np.zeros(Upad, np.int64)
    rank_starts[pads:] = starts[perm]

    # per-tile max degree -> groups of tiles with shared width
    tile_max = np.maximum(rank_cnt.reshape(T_pc, 8 * 128).max(1), 1)
    G_LIST = [4] * (T_pc // 4) + ([T_pc % 4] if T_pc % 4 else [])
    D_LIST = []
    t0 = 0
    for g in G_LIST:
        D_LIST.append(int(tile_max[t0 : t0 + g].max()))
        t0 += g

    blocks = []
    t0 = 0
    for G, D_PAD in zip(G_LIST, D_LIST):
        DE = D_PAD + 1
        COLT = 8 * DE
        r0, r1 = t0 * 1024, (t0 + G) * 1024
        nrow = r1 - r0
        sp = np.zeros((nrow, DE), np.int64)
        c0 = rank_cnt[r0:r1]
        st = rank_starts[r0:r1]
        # scatter edges: rows sorted by rank; edge k of row i at col k
        rows = np.repeat(np.arange(nrow), c0)
        cols = np.arange(rows.shape[0]) - np.repeat(
            np.cumsum(c0) - c0, c0)
        eidx = (np.repeat(st, c0) + cols)
        sp[rows, cols] = es[eidx]
        sp[:, D_PAD] = rank_uid[r0:r1]
        xgB = np.zeros((nrow, COLT), np.float32)
        xgB[:, : FIN * DE] = (
            x[sp.reshape(-1)].reshape(nrow, DE, FIN)
            .transpose(0, 2, 1).reshape(nrow, FIN * DE))
        j = np.arange(DE)[None, :]
        xgB[:, FIN * DE :] = np.where(
            j < c0[:, None], 0.0, -60000.0).astype(np.float32)
        blocks.append(np.ascontiguousarray(
            xgB.reshape(G, NCORES, 128, COLT)
            .transpose(1, 2, 0, 3)
            .reshape(NCORES, 128, G * COLT)))
        t0 += G
    xg2 = np.concatenate(blocks, axis=2)

    # output row of node-uidx u: device rows ordered (core, tile, slot)
    r = np.arange(pads, Upad)
    t, c, s = r // 1024, (r // 128) % 8, r % 128
    row_of_u = np.empty(U, np.int64)
    row_of_u[perm] = (c * T_pc + t) * 128 + s
    core_of_u = np.empty(U, np.int64)
    core_of_u[perm] = c

    return {"T_pc": T_pc, "G_LIST": tuple(G_LIST), "D_LIST": tuple(D_LIST),
            "xg2": xg2, "inv": inv, "row_of_u": row_of_u,
            "core_of_u": core_of_u}


def _const_pack(W, att_src, att_dst, gat_bias, ln_w, ln_b, lin_W, lin_b):
    W = np.ascontiguousarray(W, np.float32).reshape(FIN, HC)
    attS = np.ascontiguousarray(att_src, np.float32).reshape(HC)
    attD = np.ascontiguousarray(att_dst, np.float32).reshape(HC)
    gb = np.ascontiguousarray(gat_bias, np.float32).reshape(HC)
    cp = np.zeros((128, NC), np.float32)
    cp[:, C_ID : C_ID + 128] = np.eye(128, dtype=np.float32)
    # WbFull rows 32q+r: r<28 -> Wb row r ((h,f)=divmod(r,7)); r==28 -> gb
    wb = np.zeros((32, 128), np.float32)
    for r in range(28):
        h, f = divmod(r, FIN)
        wb[r, h * C : (h + 1) * C] = W[f, h * C : (h + 1) * C]
    wb[28, :] = gb
    for q in range(4):
        cp[32 * q : 32 * (q + 1), C_WB : C_WB + 128] = wb
    # WT4G col 32q+m: m<28 -> Wb row m transposed; m==28 -> gb
    wt = np.zeros((128, 32), np.float32)
    wt[:, :29] = wb[:29].T
    cp[:, C_WT : C_WT + 125] = np.tile(wt, (1, 4))[:, :125]
    cp[:, C_LIN : C_LIN + CLS] = np.ascontiguousarray(
        lin_W, np.float32).reshape(HC, CLS)
    cp[:, C_LNB] = np.ascontiguousarray(ln_b, np.float32).reshape(HC)
    cp[:, C_ONE] = 1.0
    cp[:, C_LNW] = np.ascontiguousarray(ln_w, np.float32).reshape(HC)
    cp[:, C_EPS] = 1e-5
    cp[:, C_E16] = 1e-16
    cp[0, C_SEL : C_SEL + CLS] = 1.0
    cp[1, C_SEL + CLS : C_SEL + 14] = 1.0
    cp[0, C_LB2 : C_LB2 + CLS] = np.ascontiguousarray(
        lin_b, np.float32).reshape(CLS)
    cp[0:2, C_O2 : C_O2 + 128] = 1.0
    cp2 = np.empty((128, 1152), np.float32)
    cp2[:, 0:896] = W.reshape(-1)[None, :]
    cp2[:, 896:1024] = attS[None, :]
    cp2[:, 1024:1152] = attD[None, :]
    return np.ascontiguousarray(np.concatenate([cp, cp2], axis=1))


def _ap(base, off_elems, dims):
    """AP with explicit free dims; dims = [[step, count], ...]."""
    return bass.AP(base.tensor, base.offset + off_elems,
                   [list(base.ap[0])] + dims)


# ---------------------------------------------------------------- program
def _build(T_pc, G_LIST, D_LIST):
    nc = bacc.Bacc("TRN2", target_bir_lowering=False, debug=False,
                   num_devices=NCORES)
    NG = len(G_LIST)
    G_MAX = max(G_LIST)
    D_MAX = max(D_LIST)
    DE_MAX = D_MAX + 1
    JF_MAX = FIN * DE_MAX
    COLT_L = [8 * (d + 1) for d in D_LIST]
    TOTC = sum(g * c for g, c in zip(G_LIST, COLT_L))
    # per-level max widths of the generic j-tree (for tile allocation)
    halves_max = []
    n = D_MAX
    while n > 1:
        halves_max.append(n // 2)
        n //= 2

    d_xg = nc.dram_tensor("xg2", [128, TOTC], F32, kind="ExternalInput")
    d_cp = nc.dram_tensor("cpack", [128, NC + 1152], F32,
                          kind="ExternalInput")
    d_out = nc.dram_tensor("probs", [128, T_pc * CLS], F32,
                           kind="ExternalOutput")

    AX = mybir.AxisListType.X
    OP = mybir.AluOpType
    ACT = mybir.ActivationFunctionType

    with tile.TileContext(nc) as tc:
        with (
            tc.tile_pool(name="const", bufs=1) as cp,
            tc.tile_pool(name="work", bufs=2) as wp,
            tc.tile_pool(name="pp_p", bufs=1, space="PSUM") as pp_p,
            tc.tile_pool(name="pp_t", bufs=2, space="PSUM") as pp_t,
            tc.tile_pool(name="pp_o", bufs=2, space="PSUM") as pp_o,
        ):
            # ---- prologue: packed const DMAs, then on-device weight prep
            CP = cp.tile([128, NC + 1152], F32, tag="CP")
            nc.sync.dma_start(out=CP[:], in_=d_cp[:, :])
            CP2 = CP[:, NC : NC + 1152]
            ident = CP[:, C_ID : C_ID + 128]

            # main-input DMAs issued up-front (in-order Sync queue)
            goff0 = [0]
            for g in range(NG):
                goff0.append(goff0[-1] + G_LIST[g] * COLT_L[g])
            xgs = {}
            for g in range(NG):
                w = G_LIST[g] * COLT_L[g]
                xgt = wp.tile([128, G_MAX * 8 * DE_MAX], F32, tag="xg")
                nc.sync.dma_start(out=xgt[:, 0:w],
                                  in_=d_xg[:, goff0[g] : goff0[g] + w])
                xgs[g] = xgt

            # As/Ad computed redundantly in every partition:
            # a78[p,(w,f,h)] = sum_c W[f,hc]*att_w[hc]
            t78 = cp.tile([128, 1792], F32, tag="t78")
            nc.gpsimd.tensor_tensor(
                out=_ap(t78[:], 0, [[896, 2], [128, FIN], [1, 128]]),
                in0=_ap(CP2, 0, [[0, 2], [128, FIN], [1, 128]]),
                in1=_ap(CP2, 896, [[128, 2], [0, FIN], [1, 128]]),
                op=OP.mult)
            a78 = cp.tile([128, 56], F32, tag="a78")
            nc.vector.tensor_reduce(
                out=_ap(a78[:], 0, [[4, 14], [1, 4]]),
                in_=_ap(t78[:], 0, [[128, 14], [32, 4], [1, 32]]),
                axis=AX, op=OP.add)
            # expanded fp16 (h,f,j) table of As (a_dst handled separately;
            # the pseudo-neighbor column j=D_PAD is masked anyway)
            AsE = cp.tile([128, H * JF_MAX], F16, tag="AsE")
            nc.scalar.activation(
                out=_ap(AsE[:], 0, [[JF_MAX, H], [DE_MAX, FIN], [1, DE_MAX]]),
                in_=_ap(a78[:], 0, [[1, H], [4, FIN], [0, DE_MAX]]),
                func=ACT.Copy)

            # linWp = lnw * linW (cols 7:14 of lin2 region, inside CP)
            nc.vector.tensor_scalar(
                out=CP[:, C_LIN + CLS : C_LIN + 2 * CLS],
                in0=CP[:, C_LIN : C_LIN + CLS],
                scalar1=CP[:, C_LNW : C_LNW + 1], scalar2=None, op0=OP.mult)
            # Wbc[32q+m, k] = (Wb@linWp')[m,k] (m<28) / (gb@linWp')[k] (m=28)
            ps_w = pp_p.tile([125, CLS], F32, tag="psw")
            nc.tensor.matmul(
                out=ps_w[:], lhsT=CP[:, C_WT : C_WT + 125],
                rhs=CP[:, C_LIN + CLS : C_LIN + 2 * CLS],
                start=True, stop=True)
            WbF = cp.tile([128, 136], F16, tag="WbF")
            nc.scalar.activation(out=WbF[:, 0:128],
                                 in_=CP[:, C_WB : C_WB + 128], func=ACT.Copy)
            nc.scalar.activation(out=WbF[0:125, 128:135], in_=ps_w[:],
                                 func=ACT.Copy)
            id16 = cp.tile([128, 128], F16, tag="id16")
            nc.scalar.activation(out=id16[:], in_=ident, func=ACT.Copy)
            # wsum col: -(row sum of [Wb;gb]) / HC  -> matmul emits -mean(o)
            ws = cp.tile([128, 1], F32, tag="ws")
            nc.vector.tensor_reduce(
                out=ws[:], in_=CP[:, C_WB : C_WB + 128], axis=AX, op=OP.add)
            nc.scalar.activation(out=WbF[:, 135:136], in_=ws[:],
                                 func=ACT.Copy, scale=-1.0 / HC)
            # [lbp | colsum] broadcast rows
            ps_a = pp_p.tile([2, 14], F32, tag="psa")
            nc.tensor.matmul(
                out=ps_a[:], lhsT=CP[:, C_LNB : C_LNB + 2],
                rhs=CP[:, C_LIN : C_LIN + 14],
                start=True, stop=True)
            z2a = cp.tile([2, 14], F32, tag="z2a")
            nc.vector.tensor_tensor(out=z2a[:], in0=ps_a[:],
                                    in1=CP[0:2, C_SEL : C_SEL + 14],
                                    op=OP.mult)
            z2 = cp.tile([2, 14], F32, tag="z2")
            nc.vector.tensor_tensor(out=z2[:], in0=z2a[:],
                                    in1=CP[0:2, C_LB2 : C_LB2 + 14],
                                    op=OP.add)
            ps_b = pp_p.tile([128, 14], F32, tag="psb")
            nc.tensor.matmul(
                out=ps_b[:], lhsT=CP[0:2, C_O2 : C_O2 + 128],
                rhs=z2[:], start=True, stop=True)
            LC = cp.tile([128, 14], F32, tag="LC")
            nc.scalar.copy(out=LC[:], in_=ps_b[:])

            # ---- main loop: NG groups in a software pipeline
            toff = [0]
            for g in range(NG):
                toff.append(toff[-1] + G_LIST[g])

            def phaseA(g):
                G, D_PAD = G_LIST[g], D_LIST[g]
                t0 = toff[g]
                DE = D_PAD + 1
                COLT = 8 * DE
                JF = FIN * DE
                JH = H * DE
                MJF = FIN * D_PAD
                O_MK = JF
                NB = -(-G // 3)
                TH = G * H
                THF = TH * FIN
                xg = xgs[g]
                # fp16 cast, one op per tile for pipelining
                xb = wp.tile([128, G_MAX * 8 * DE_MAX], F16, tag="xb")
                for t in range(G):
                    nc.scalar.activation(
                        out=xb[:, t * COLT : (t + 1) * COLT],
                        in_=xg[:, t * COLT : (t + 1) * COLT], func=ACT.Copy)

                # a_dst: ad[s,(t,h)] = sum_f xslot*Ad  (from fp32 xg)
                pd = wp.tile([128, G_MAX * H * FIN], F32, tag="pd")
                nc.vector.tensor_tensor(
                    out=_ap(pd[:], 0, [[H * FIN, G], [FIN, H], [1, FIN]]),
                    in0=_ap(xg[:], D_PAD, [[COLT, G], [0, H], [DE, FIN]]),
                    in1=_ap(a78[:], 28, [[0, G], [1, H], [4, FIN]]),
                    op=OP.mult)
                adt = wp.tile([128, G_MAX * H], F32, tag="adt")
                nc.vector.tensor_reduce(
                    out=_ap(adt[:], 0, [[H, G], [1, H]]),
                    in_=_ap(pd[:], 0, [[H * FIN, G], [FIN, H], [1, FIN]]),
                    axis=AX, op=OP.add)
                adb = wp.tile([128, G_MAX * H], F16, tag="adb")
                nc.scalar.activation(out=adb[:, 0 : G * H],
                                     in_=adt[:, 0 : G * H], func=ACT.Copy)
                # a_src products: pa[s,(t,h,f,j)] = xb * AsE  (fp16 2x)
                pa = wp.tile([128, G_MAX * H * JF_MAX], F16, tag="pa")
                for t in range(G):
                    eng = nc.gpsimd if (D_PAD <= 20 and t % 2 == 1) else nc.vector
                    eng.tensor_tensor(
                        out=_ap(pa[:], t * H * JF,
                                [[JF, H], [DE, FIN], [1, DE]]),
                        in0=_ap(xb[:], t * COLT,
                                [[0, H], [DE, FIN], [1, DE]]),
                        in1=_ap(AsE[:], 0,
                                [[JF_MAX, H], [DE_MAX, FIN], [1, DE]]),
                        op=OP.mult)
                # tree-reduce over f (7 = 3+3+1): sE[s,(t,h,j)]
                q3 = wp.tile([128, G_MAX * H * 3 * DE_MAX], F16, tag="q3")
                nc.vector.tensor_tensor(
                    out=_ap(q3[:], 0, [[3 * DE, TH], [DE, 3], [1, DE]]),
                    in0=_ap(pa[:], 0, [[JF, TH], [DE, 3], [1, DE]]),
                    in1=_ap(pa[:], 3 * DE, [[JF, TH], [DE, 3], [1, DE]]),
                    op=OP.add)
                r1 = wp.tile([128, G_MAX * H * DE_MAX], F16, tag="r1")
                nc.vector.tensor_tensor(
                    out=_ap(r1[:], 0, [[DE, TH], [1, DE]]),
                    in0=_ap(q3[:], 0, [[3 * DE, TH], [1, DE]]),
                    in1=_ap(q3[:], DE, [[3 * DE, TH], [1, DE]]),
                    op=OP.add)
                r2 = wp.tile([128, G_MAX * H * DE_MAX], F16, tag="r2")
                nc.vector.tensor_tensor(
                    out=_ap(r2[:], 0, [[DE, TH], [1, DE]]),
                    in0=_ap(r1[:], 0, [[DE, TH], [1, DE]]),
                    in1=_ap(q3[:], 2 * DE, [[3 * DE, TH], [1, DE]]),
                    op=OP.add)
                sE = wp.tile([128, G_MAX * H * DE_MAX], F16, tag="sE")
                nc.vector.tensor_tensor(
                    out=_ap(sE[:], 0, [[DE, TH], [1, DE]]),
                    in0=_ap(r2[:], 0, [[DE, TH], [1, DE]]),
                    in1=_ap(pa[:], 6 * DE, [[JF, TH], [1, DE]]),
                    op=OP.add)
                # + mask, + a_dst bcast, leaky (gpsimd), then exp
                sF = wp.tile([128, G_MAX * H * DE_MAX], F16, tag="sF")
                (nc.vector if D_PAD <= 20 else nc.gpsimd).tensor_tensor(
                    out=_ap(sF[:], 0, [[JH, G], [DE, H], [1, DE]]),
                    in0=_ap(sE[:], 0, [[JH, G], [DE, H], [1, DE]]),
                    in1=_ap(xb[:], O_MK, [[COLT, G], [0, H], [1, DE]]),
                    op=OP.add)
                sG = wp.tile([128, G_MAX * H * DE_MAX], F16, tag="sG")
                (nc.vector if D_PAD <= 20 else nc.gpsimd).tensor_tensor(
                    out=_ap(sG[:], 0, [[JH, G], [DE, H], [1, DE]]),
                    in0=_ap(sF[:], 0, [[JH, G], [DE, H], [1, DE]]),
                    in1=_ap(adb[:], 0, [[H, G], [1, H], [0, DE]]),
                    op=OP.add)
                ezl = wp.tile([128, G_MAX * H * DE_MAX], F16, tag="ezl")
                nc.vector.scalar_tensor_tensor(
                    out=ezl[:, 0 : TH * DE], in0=sG[:, 0 : TH * DE],
                    scalar=NEG, in1=sG[:, 0 : TH * DE],
                    op0=OP.mult, op1=OP.max)
                mx = wp.tile([128, G_MAX * H], F16, tag="mx")
                nc.vector.tensor_reduce(
                    out=_ap(mx[:], 0, [[H, G], [1, H]]),
                    in_=_ap(ezl[:], 0, [[JH, G], [DE, H], [1, DE]]),
                    axis=AX, op=OP.max)
                ezm = wp.tile([128, G_MAX * H * DE_MAX], F16, tag="ezm")
                nc.gpsimd.tensor_tensor(
                    out=_ap(ezm[:], 0, [[JH, G], [DE, H], [1, DE]]),
                    in0=_ap(ezl[:], 0, [[JH, G], [DE, H], [1, DE]]),
                    in1=_ap(mx[:], 0, [[H, G], [1, H], [0, DE]]),
                    op=OP.subtract)
                ez = wp.tile([128, G_MAX * H * DE_MAX], F16, tag="ez")
                nc.scalar.activation(out=ez[:, 0 : TH * DE],
                                     in_=ezm[:, 0 : TH * DE], func=ACT.Exp)

                # denominators (fp32)
                den = wp.tile([128, G_MAX * H], F32, tag="den")
                nc.vector.tensor_reduce(
                    out=_ap(den[:], 0, [[H, G], [1, H]]),
                    in_=_ap(ez[:], 0, [[JH, G], [DE, H], [1, DE]]),
                    axis=AX, op=OP.add)

                # messages: pm[s,(t,h,f,j)] = ez * xb  (fp16 2x, j<D_PAD)
                pm = wp.tile([128, G_MAX * H * FIN * D_MAX], F16, tag="pm")
                for t in range(G):
                    eng = nc.gpsimd if (D_PAD <= 20 and t % 2 == 0) else nc.vector
                    eng.tensor_tensor(
                        out=_ap(pm[:], t * H * MJF,
                                [[MJF, H], [D_PAD, FIN], [1, D_PAD]]),
                        in0=_ap(ez[:], t * JH,
                                [[DE, H], [0, FIN], [1, D_PAD]]),
                        in1=_ap(xb[:], t * COLT,
                                [[0, H], [DE, FIN], [1, D_PAD]]),
                        op=OP.mult)
                # generic pairwise tree-reduce over j; odd leftovers folded
                # in at the end; small levels accumulate in fp32
                cur, stride, ncur = pm, D_PAD, D_PAD
                leftovers = []
                lvl = 0
                while ncur > 1:
                    half = ncur // 2
                    if ncur % 2:
                        leftovers.append((cur, stride, ncur - 1))
                    dt = F16 if half >= 4 else F32
                    hm = halves_max[lvl] if lvl < len(halves_max) else half
                    nxt = wp.tile([128, G_MAX * H * FIN * max(hm, half)],
                                  dt, tag=f"jt{lvl}")
                    nc.vector.tensor_tensor(
                        out=_ap(nxt[:], 0, [[half, THF], [1, half]]),
                        in0=_ap(cur[:], 0, [[stride, THF], [1, half]]),
                        in1=_ap(cur[:], half, [[stride, THF], [1, half]]),
                        op=OP.add)
                    cur, stride, ncur = nxt, half, half
                    lvl += 1
                for li, (buf, st, off) in enumerate(leftovers):
                    nxt = wp.tile([128, G_MAX * H * FIN], F32, tag=f"jl{li}")
                    nc.vector.tensor_tensor(
                        out=_ap(nxt[:], 0, [[1, THF]]),
                        in0=_ap(cur[:], 0, [[stride, THF]]),
                        in1=_ap(buf[:], off, [[st, THF]]),
                        op=OP.add)
                    cur, stride = nxt, 1
                SD = cur

                # normalize: Sn = SD/(den+eps), ones col 28 per tile
                rd = wp.tile([128, G_MAX * H], F32, tag="rd")
                nc.scalar.activation(out=rd[:, 0 : G * H],
                                     in_=den[:, 0 : G * H], func=ACT.Copy,
                                     bias=1e-16)
                nc.vector.reciprocal(out=rd[:, 0 : G * H],
                                     in_=rd[:, 0 : G * H])
                Sn = wp.tile([128, G_MAX * 32], F16, tag="Sn")
                nc.gpsimd.memset(_ap(Sn[:], 28, [[32, G]]), 1.0)
                nc.vector.tensor_tensor(
                    out=_ap(Sn[:], 0, [[32, G], [FIN, H], [1, FIN]]),
                    in0=_ap(SD[:], 0, [[H * FIN, G], [FIN, H], [1, FIN]]),
                    in1=_ap(rd[:], 0, [[H, G], [1, H], [0, FIN]]),
                    op=OP.mult)

                # transpose 3 tiles per PE pass; quadrant-aligned SnT
                SnTs = []
                for b in range(NB):
                    w = min(96, G * 32 - b * 96)
                    ps_t = pp_t.tile([96, 128], F16, tag="pst")
                    nc.tensor.transpose(out=ps_t[0:w, :],
                                        in_=Sn[:, b * 96 : b * 96 + w],
                                        identity=id16)
                    SnT = wp.tile([96, 128], F16, tag=f"snt{b}")
                    nc.scalar.activation(out=SnT[0:w, :], in_=ps_t[0:w, :],
                                         func=ACT.Copy)
                    SnTs.append(SnT)

                return SnTs

            def phaseB(g, SnTs):
                G, D_PAD = G_LIST[g], D_LIST[g]
                t0 = toff[g]
                # per tile: [o | q | -mu] = SnT.T @ WbFull, then LN + logits
                vs = wp.tile([128, G_MAX], F32, tag="vs")
                nm = wp.tile([128, G_MAX], F32, tag="nm")
                u_sb = wp.tile([128, G_MAX * CLS], F32, tag="u")
                lg = wp.tile([128, G_MAX * CLS], F32, tag="lg")
                for t in range(G):
                    b, tb = divmod(t, 3)
                    base = tb * 32
                    ps_o = pp_o.tile([128, 136], F32, tag="pso")
                    nc.tensor.matmul(
                        out=ps_o[:],
                        lhsT=SnTs[b][base : base + 29, :],
                        rhs=WbF[base : base + 29, :],
                        start=True, stop=True)
                    nc.vector.tensor_scalar(
                        out=nm[:, t : t + 1], in0=ps_o[:, 135:136],
                        scalar1=1.0, scalar2=None, op0=OP.mult)
                    sqt = wp.tile([128, HC], F32, tag="sqt")
                    nc.scalar.activation(
                        out=sqt[:], in_=ps_o[:, 0:HC], func=ACT.Square,
                        bias=nm[:, t : t + 1], accum_out=vs[:, t : t + 1])
                    nc.vector.scalar_tensor_tensor(
                        out=u_sb[:, t * CLS : (t + 1) * CLS],
                        in0=LC[:, CLS:14], scalar=ps_o[:, 135:136],
                        in1=ps_o[:, HC : HC + CLS],
                        op0=OP.mult, op1=OP.add)
                nc.scalar.activation(out=vs[:, 0:G], in_=vs[:, 0:G],
                                     func=ACT.Ln, scale=1.0 / HC,
                                     bias=CP[:, C_EPS : C_EPS + 1])
                nc.scalar.activation(out=vs[:, 0:G], in_=vs[:, 0:G],
                                     func=ACT.Exp, scale=-0.5)
                nc.vector.tensor_tensor(
                    out=_ap(lg[:], 0, [[CLS, G], [1, CLS]]),
                    in0=_ap(u_sb[:], 0, [[CLS, G], [1, CLS]]),
                    in1=_ap(vs[:], 0, [[1, G], [0, CLS]]), op=OP.mult)
                nc.vector.tensor_tensor(
                    out=_ap(lg[:], 0, [[CLS, G], [1, CLS]]),
                    in0=_ap(lg[:], 0, [[CLS, G], [1, CLS]]),
                    in1=_ap(LC[:], 0, [[0, G], [1, CLS]]), op=OP.add)
                nc.scalar.activation(out=lg[:, 0 : G * CLS],
                                     in_=lg[:, 0 : G * CLS], func=ACT.Exp)
                se = wp.tile([128, G_MAX], F32, tag="se")
                nc.vector.tensor_reduce(
                    out=se[:, 0:G],
                    in_=_ap(lg[:], 0, [[CLS, G], [1, CLS]]),
                    axis=AX, op=OP.add)
                nc.vector.reciprocal(out=se[:, 0:G], in_=se[:, 0:G])
                po = wp.tile([128, G_MAX * CLS], F32, tag="po")
                nc.gpsimd.tensor_tensor(
                    out=_ap(po[:], 0, [[CLS, G], [1, CLS]]),
                    in0=_ap(lg[:], 0, [[CLS, G], [1, CLS]]),
                    in1=_ap(se[:], 0, [[1, G], [0, CLS]]), op=OP.mult)
                nc.sync.dma_start(
                    out=d_out[:, t0 * CLS : (t0 + G) * CLS],
                    in_=po[:, 0 : G * CLS])


            # software pipeline: A0 A1 B0 A2 B1 ... B_last — each group's
            # psum-dependent tail sits after the next group's front half in
            # every engine queue, so engines never stall at group seams
            snts = {0: phaseA(0)}
            if NG > 1:
                snts[1] = phaseA(1)
            phaseB(0, snts[0])
            for g in range(2, NG):
                snts[g] = phaseA(g)
                phaseB(g - 1, snts[g - 1])
            if NG > 1:
                phaseB(NG - 1, snts[NG - 1])
    nc.compile()
    return nc


_CACHE = {}


def _program(T_pc, G_LIST, D_LIST):
    key = (T_pc, G_LIST, D_LIST)
    if key not in _CACHE:
        _CACHE[key] = _build(T_pc, G_LIST, D_LIST)
    return _CACHE[key]


# ---------------------------------------------------------------- entry
def kernel(x, edge_weight, W, att_src, att_dst, gat_bias, ln_w, ln_b,
           lin_W, lin_b, edge_index, ids):
    prep = _preprocess(np.asarray(x), np.asarray(edge_index),
                       np.asarray(ids))
    T_pc = prep["T_pc"]
    nc = _program(T_pc, prep["G_LIST"], prep["D_LIST"])
    cpack = _const_pack(W, att_src, att_dst, gat_bias, ln_w, ln_b,
                        lin_W, lin_b)

    in_maps = [{"xg2": prep["xg2"][c], "cpack": cpack}
               for c in range(NCORES)]

    if os.environ.get("KERNEL_SIM"):
        from concourse.bass_interp import CoreSim

        outs = []
        ncores = int(os.environ.get("KERNEL_SIM_CORES", "1"))
        for c in range(ncores):
            sim = CoreSim(nc, require_finite=False, require_nnan=False)
            for k, v in in_maps[c].items():
                sim.tensor(k)[:] = v
            sim.simulate()
            outs.append(sim.tensor("probs").copy())
        arr = np.stack(outs + [np.zeros_like(outs[0])] * (NCORES - ncores))
    else:
        trace = bool(int(os.environ.get("KERNEL_TRACE", "0")))
        res = bass_utils.run_bass_kernel_spmd(
            nc, in_maps, core_ids=list(range(NCORES)), trace=trace)
        if trace and res.exec_time_ns is not None:
            print(f"HW exec time: {res.exec_time_ns} ns")
        arr = np.stack([res.results[c]["probs"] for c in range(NCORES)])

    full = (arr.reshape(NCORES, 128, T_pc, CLS)
            .transpose(0, 2, 1, 3)
            .reshape(NCORES * T_pc * 128, CLS))
    return np.ascontiguousarray(
        full[prep["row_of_u"]][prep["inv"]], np.float32)


# revision 38
# speedup vs baseline: 1.0609x; 1.0441x over previous
"""GAT node-classification kernel for Trainium2 (8 NeuronCores, SPMD).

Strategy (dst-node graph partitioning per the sharding hint):
  - Only destination nodes appearing in `ids` affect the output; edges into
    other nodes are dead code and are dropped.
  - Destination nodes are sorted by in-degree and packed 128 per tile in
    rank order, dealt so all 8 cores share the same per-tile degree
    profile.  Tiles are batched into pipeline groups whose edge-slot width
    is the group's max degree (+1 pseudo-neighbor column carrying the
    node's own features for a_dst) — the attention/message work scales
    with the group width instead of the global max degree.
  - Per-edge attention logits are 7-term feature contractions on the
    Vector/GpSimd engines in fp16 (2-byte DVE fast modes); reductions are
    pairwise trees of tensor_tensor adds whose last levels accumulate in
    fp32.  A per-(tile,head) max is subtracted before exp (exact softmax
    shift) so fp16 exp inputs stay in range.  Weighted message sums stay
    in the rank-7 feature basis (sum(alpha*(x@W)) == (sum(alpha*x))@W).
  - Per tile the normalized sums (plus an appended ones column carrying
    the GAT bias and folded classifier bias) are transposed once on the
    PE (3 tiles per pass, quadrant-aligned) and hit a single fp16 matmul
    with rhs = [Wb;gb | Wb@linWp' | -rowsum/HC], yielding the GAT output
    o, the classifier projection q, and -mean(o) in one pass.  LayerNorm's
    affine transform is folded into the classifier weights; 1/std is
    applied after the matmul; rstd = exp(-0.5*ln(var)) so every
    activation shares one hardware function table.
  - All device inputs are packed host-side so each SBUF partition's data
    is contiguous in DRAM (one descriptor set per transfer); input DMAs
    are issued up-front on the in-order Sync queue; As/Ad are computed
    redundantly in every partition to avoid a DRAM broadcast roundtrip.

The host does no floating-point arithmetic on tensor values: it only
filters/sorts/permutes (sharding layout, gathers, block-diagonal placement
of W) and builds 0/1, eye, ones and 0/-60000 masks; all float math and all
dtype conversion runs on the NeuronCores.
"""
import os
import sys

sys.path.insert(0, "/opt/trn_rl_repo")

import numpy as np

import concourse.bass as bass
import concourse.bacc as bacc
import concourse.mybir as mybir
import concourse.tile as tile
from concourse import bass_utils
import concourse.bacc as _bacc_mod
import concourse.hw_specs as _hw_specs

_PIN_SET = "natural_log_exp_and_others"
_orig_get_tables = _hw_specs.get_activation_tables


def _pinned_tables(arch):
    """Route every activation to one table set (exp/ln/square/copy all
    coexist there) so the kernel pays a single ACT_TABLE_LOAD."""
    tabs = _orig_get_tables(arch)
    if _PIN_SET in tabs:
        tabs = {k: (v if k == _PIN_SET else set()) for k, v in tabs.items()}
    return tabs


_bacc_mod.get_activation_tables = _pinned_tables

N = 100000
FIN = 7
H = 4
C = 32
HC = H * C  # 128
CLS = 7
NEG = 0.2
NCORES = 8
F32 = mybir.dt.float32
F32R = mybir.dt.float32r
F16 = mybir.dt.float16

# const-pack column offsets (cursor-built)
_cur = 0


def _adv(w):
    global _cur
    o = _cur
    _cur += w
    return o


C_ID = _adv(128)     # identity [128,128]
C_WB = _adv(136)     # [Wb;gb] quadrant-replicated (128) | Wbc (7) | -rowsum/HC
C_WT = _adv(128)     # WT4G [128,125] (pad 3)
C_LIN = _adv(14)     # linW | (linWp written by device)
C_LNB = _adv(1)      # lnb column
C_ONE = _adv(1)      # ones column (adjacent: lhsT [128,2])
C_LNW = _adv(1)      # lnw column
C_EPS = _adv(1)      # 1e-5 column
C_E16 = _adv(1)      # 1e-16 column
C_SEL = _adv(14)     # rows 0:2 = keep-mask for [lbp | colsum] assembly
C_LB2 = _adv(14)     # row 0 cols 0:7 = lin_b, else 0
C_O2 = _adv(128)     # rows 0:2 = ones
NC = _cur + (-_cur % 8)


# ---------------------------------------------------------------- host prep
def _preprocess(x, edge_index, ids):
    src = np.asarray(edge_index[0], dtype=np.int64)
    dst = np.asarray(edge_index[1], dtype=np.int64)
    ids = np.asarray(ids, dtype=np.int64)
    x = np.asarray(x, np.float32)

    uids, inv = np.unique(ids, return_inverse=True)
    U = uids.shape[0]
    mark = np.full(N, -1, np.int64)
    mark[uids] = np.arange(U)

    dstc = mark[dst]
    keep = dstc >= 0
    es = src[keep]
    ed = dstc[keep]
    order = np.argsort(ed, kind="stable")
    es = es[order]
    ed = ed[order]
    cnt = np.bincount(ed, minlength=U).astype(np.int64)
    starts = np.zeros(U + 1, np.int64)
    np.cumsum(cnt, out=starts[1:])

    T_need = -(-U // 128)
    T_pc = -(-T_need // NCORES)
    T_pc += T_pc % 2
    T_tot = T_pc * NCORES
    Upad = T_tot * 128
    pads = Upad - U

    # degree-ascending rank: pad slots first, then nodes sorted by degree.
    # rank r -> tile t=r//1024, core c=(r//128)%8, slot s=r%128, so every
    # core sees the same per-tile degree profile.
    perm = np.argsort(cnt, kind="stable")
    rank_cnt = np.zeros(Upad, np.int64)
    rank_cnt[pads:] = cnt[perm]
    rank_uid = np.zeros(Upad, np.int64)
    rank_uid[pads:] = uids[perm]
    rank_uidx = np.full(Upad, -1, np.int64)
    rank_uidx[pads:] = perm
    rank_starts = np.zeros(Upad, np.int64)
    rank_starts[pads:] = starts[perm]

    # per-tile max degree -> groups of tiles with shared width
    tile_max = np.maximum(rank_cnt.reshape(T_pc, 8 * 128).max(1), 1)
    G_LIST = [4] * (T_pc // 4) + ([T_pc % 4] if T_pc % 4 else [])
    D_LIST = []
    t0 = 0
    for g in G_LIST:
        D_LIST.append(int(tile_max[t0 : t0 + g].max()))
        t0 += g

    blocks = []
    t0 = 0
    for G, D_PAD in zip(G_LIST, D_LIST):
        DE = D_PAD + 1
        COLT = 8 * DE
        r0, r1 = t0 * 1024, (t0 + G) * 1024
        nrow = r1 - r0
        sp = np.zeros((nrow, DE), np.int64)
        c0 = rank_cnt[r0:r1]
        st = rank_starts[r0:r1]
        # scatter edges: rows sorted by rank; edge k of row i at col k
        rows = np.repeat(np.arange(nrow), c0)
        cols = np.arange(rows.shape[0]) - np.repeat(
            np.cumsum(c0) - c0, c0)
        eidx = (np.repeat(st, c0) + cols)
        sp[rows, cols] = es[eidx]
        sp[:, D_PAD] = rank_uid[r0:r1]
        xgB = np.zeros((nrow, COLT), np.float32)
        xgB[:, : FIN * DE] = (
            x[sp.reshape(-1)].reshape(nrow, DE, FIN)
            .transpose(0, 2, 1).reshape(nrow, FIN * DE))
        j = np.arange(DE)[None, :]
        xgB[:, FIN * DE :] = np.where(
            j < c0[:, None], 0.0, -60000.0).astype(np.float32)
        blocks.append(np.ascontiguousarray(
            xgB.reshape(G, NCORES, 128, COLT)
            .transpose(1, 2, 0, 3)
            .reshape(NCORES, 128, G * COLT)))
        t0 += G
    xg2 = np.concatenate(blocks, axis=2)

    # output row of node-uidx u: device rows ordered (core, tile, slot)
    r = np.arange(pads, Upad)
    t, c, s = r // 1024, (r // 128) % 8, r % 128
    row_of_u = np.empty(U, np.int64)
    row_of_u[perm] = (c * T_pc + t) * 128 + s
    core_of_u = np.empty(U, np.int64)
    core_of_u[perm] = c

    return {"T_pc": T_pc, "G_LIST": tuple(G_LIST), "D_LIST": tuple(D_LIST),
            "xg2": xg2, "inv": inv, "row_of_u": row_of_u,
            "core_of_u": core_of_u}


def _const_pack(W, att_src, att_dst, gat_bias, ln_w, ln_b, lin_W, lin_b):
    W = np.ascontiguousarray(W, np.float32).reshape(FIN, HC)
    attS = np.ascontiguousarray(att_src, np.float32).reshape(HC)
    attD = np.ascontiguousarray(att_dst, np.float32).reshape(HC)
    gb = np.ascontiguousarray(gat_bias, np.float32).reshape(HC)
    cp = np.zeros((128, NC), np.float32)
    cp[:, C_ID : C_ID + 128] = np.eye(128, dtype=np.float32)
    # WbFull rows 32q+r: r<28 -> Wb row r ((h,f)=divmod(r,7)); r==28 -> gb
    wb = np.zeros((32, 128), np.float32)
    for r in range(28):
        h, f = divmod(r, FIN)
        wb[r, h * C : (h + 1) * C] = W[f, h * C : (h + 1) * C]
    wb[28, :] = gb
    for q in range(4):
        cp[32 * q : 32 * (q + 1), C_WB : C_WB + 128] = wb
    # WT4G col 32q+m: m<28 -> Wb row m transposed; m==28 -> gb
    wt = np.zeros((128, 32), np.float32)
    wt[:, :29] = wb[:29].T
    cp[:, C_WT : C_WT + 125] = np.tile(wt, (1, 4))[:, :125]
    cp[:, C_LIN : C_LIN + CLS] = np.ascontiguousarray(
        lin_W, np.float32).reshape(HC, CLS)
    cp[:, C_LNB] = np.ascontiguousarray(ln_b, np.float32).reshape(HC)
    cp[:, C_ONE] = 1.0
    cp[:, C_LNW] = np.ascontiguousarray(ln_w, np.float32).reshape(HC)
    cp[:, C_EPS] = 1e-5
    cp[:, C_E16] = 1e-16
    cp[0, C_SEL : C_SEL + CLS] = 1.0
    cp[1, C_SEL + CLS : C_SEL + 14] = 1.0
    cp[0, C_LB2 : C_LB2 + CLS] = np.ascontiguousarray(
        lin_b, np.float32).reshape(CLS)
    cp[0:2, C_O2 : C_O2 + 128] = 1.0
    cp2 = np.empty((128, 1152), np.float32)
    cp2[:, 0:896] = W.reshape(-1)[None, :]
    cp2[:, 896:1024] = attS[None, :]
    cp2[:, 1024:1152] = attD[None, :]
    return np.ascontiguousarray(np.concatenate([cp, cp2], axis=1))


def _ap(base, off_elems, dims):
    """AP with explicit free dims; dims = [[step, count], ...]."""
    return bass.AP(base.tensor, base.offset + off_elems,
                   [list(base.ap[0])] + dims)


# ---------------------------------------------------------------- program
def _build(T_pc, G_LIST, D_LIST):
    nc = bacc.Bacc("TRN2", target_bir_lowering=False, debug=False,
                   num_devices=NCORES)
    NG = len(G_LIST)
    G_MAX = max(G_LIST)
    D_MAX = max(D_LIST)
    DE_MAX = D_MAX + 1
    JF_MAX = FIN * DE_MAX
    COLT_L = [8 * (d + 1) for d in D_LIST]
    TOTC = sum(g * c for g, c in zip(G_LIST, COLT_L))
    # per-level max widths of the generic j-tree (for tile allocation)
    halves_max = []
    n = D_MAX
    while n > 1:
        halves_max.append(n // 2)
        n //= 2

    d_xg = nc.dram_tensor("xg2", [128, TOTC], F32, kind="ExternalInput")
    d_cp = nc.dram_tensor("cpack", [128, NC + 1152], F32,
                          kind="ExternalInput")
    d_out = nc.dram_tensor("probs", [128, T_pc * CLS], F32,
                           kind="ExternalOutput")

    AX = mybir.AxisListType.X
    OP = mybir.AluOpType
    ACT = mybir.ActivationFunctionType

    with tile.TileContext(nc) as tc:
        with (
            tc.tile_pool(name="const", bufs=1) as cp,
            tc.tile_pool(name="work", bufs=2) as wp,
            tc.tile_pool(name="pp_p", bufs=1, space="PSUM") as pp_p,
            tc.tile_pool(name="pp_t", bufs=2, space="PSUM") as pp_t,
            tc.tile_pool(name="pp_o", bufs=2, space="PSUM") as pp_o,
        ):
            # ---- prologue: packed const DMAs, then on-device weight prep
            CP = cp.tile([128, NC + 1152], F32, tag="CP")
            # W/att replicas land first: they gate the As/Ad -> AsE chain
            nc.sync.dma_start(out=CP[:, NC : NC + 1152],
                              in_=d_cp[:, NC : NC + 1152])
            nc.sync.dma_start(out=CP[:, 0:NC], in_=d_cp[:, 0:NC])
            CP2 = CP[:, NC : NC + 1152]
            ident = CP[:, C_ID : C_ID + 128]

            # main-input DMAs issued up-front (in-order Sync queue)
            goff0 = [0]
            for g in range(NG):
                goff0.append(goff0[-1] + G_LIST[g] * COLT_L[g])
            xgs = {}
            for g in range(NG):
                w = G_LIST[g] * COLT_L[g]
                xgt = wp.tile([128, G_MAX * 8 * DE_MAX], F32, tag="xg")
                nc.sync.dma_start(out=xgt[:, 0:w],
                                  in_=d_xg[:, goff0[g] : goff0[g] + w])
                xgs[g] = xgt

            # As/Ad computed redundantly in every partition:
            # a78[p,(w,f,h)] = sum_c W[f,hc]*att_w[hc]
            t78 = cp.tile([128, 1792], F32, tag="t78")
            nc.vector.tensor_tensor(
                out=_ap(t78[:], 0, [[896, 2], [128, FIN], [1, 128]]),
                in0=_ap(CP2, 0, [[0, 2], [128, FIN], [1, 128]]),
                in1=_ap(CP2, 896, [[128, 2], [0, FIN], [1, 128]]),
                op=OP.mult)
            a78 = cp.tile([128, 56], F32, tag="a78")
            nc.vector.tensor_reduce(
                out=_ap(a78[:], 0, [[4, 14], [1, 4]]),
                in_=_ap(t78[:], 0, [[128, 14], [32, 4], [1, 32]]),
                axis=AX, op=OP.add)
            # expanded fp16 (h,f,j) table of As (a_dst handled separately;
            # the pseudo-neighbor column j=D_PAD is masked anyway)
            AsE = cp.tile([128, H * JF_MAX], F16, tag="AsE")
            nc.scalar.activation(
                out=_ap(AsE[:], 0, [[JF_MAX, H], [DE_MAX, FIN], [1, DE_MAX]]),
                in_=_ap(a78[:], 0, [[1, H], [4, FIN], [0, DE_MAX]]),
                func=ACT.Copy)

            # linWp = lnw * linW (cols 7:14 of lin2 region, inside CP)
            nc.vector.tensor_scalar(
                out=CP[:, C_LIN + CLS : C_LIN + 2 * CLS],
                in0=CP[:, C_LIN : C_LIN + CLS],
                scalar1=CP[:, C_LNW : C_LNW + 1], scalar2=None, op0=OP.mult)
            # Wbc[32q+m, k] = (Wb@linWp')[m,k] (m<28) / (gb@linWp')[k] (m=28)
            ps_w = pp_p.tile([125, CLS], F32, tag="psw")
            nc.tensor.matmul(
                out=ps_w[:], lhsT=CP[:, C_WT : C_WT + 125],
                rhs=CP[:, C_LIN + CLS : C_LIN + 2 * CLS],
                start=True, stop=True)
            WbF = cp.tile([128, 136], F16, tag="WbF")
            nc.scalar.activation(out=WbF[:, 0:128],
                                 in_=CP[:, C_WB : C_WB + 128], func=ACT.Copy)
            nc.scalar.activation(out=WbF[0:125, 128:135], in_=ps_w[:],
                                 func=ACT.Copy)
            id16 = cp.tile([128, 128], F16, tag="id16")
            nc.scalar.activation(out=id16[:], in_=ident, func=ACT.Copy)
            # wsum col: -(row sum of [Wb;gb]) / HC  -> matmul emits -mean(o)
            ws = cp.tile([128, 1], F32, tag="ws")
            nc.vector.tensor_reduce(
                out=ws[:], in_=CP[:, C_WB : C_WB + 128], axis=AX, op=OP.add)
            nc.scalar.activation(out=WbF[:, 135:136], in_=ws[:],
                                 func=ACT.Copy, scale=-1.0 / HC)
            # [lbp | colsum] broadcast rows
            ps_a = pp_p.tile([2, 14], F32, tag="psa")
            nc.tensor.matmul(
                out=ps_a[:], lhsT=CP[:, C_LNB : C_LNB + 2],
                rhs=CP[:, C_LIN : C_LIN + 14],
                start=True, stop=True)
            z2a = cp.tile([2, 14], F32, tag="z2a")
            nc.vector.tensor_tensor(out=z2a[:], in0=ps_a[:],
                                    in1=CP[0:2, C_SEL : C_SEL + 14],
                                    op=OP.mult)
            z2 = cp.tile([2, 14], F32, tag="z2")
            nc.vector.tensor_tensor(out=z2[:], in0=z2a[:],
                                    in1=CP[0:2, C_LB2 : C_LB2 + 14],
                                    op=OP.add)
            ps_b = pp_p.tile([128, 14], F32, tag="psb")
            nc.tensor.matmul(
                out=ps_b[:], lhsT=CP[0:2, C_O2 : C_O2 + 128],
                rhs=z2[:], start=True, stop=True)
            LC = cp.tile([128, 14], F32, tag="LC")
            nc.scalar.copy(out=LC[:], in_=ps_b[:])

            # ---- main loop: NG groups in a software pipeline
            toff = [0]
            for g in range(NG):
                toff.append(toff[-1] + G_LIST[g])

            def phaseA(g):
                G, D_PAD = G_LIST[g], D_LIST[g]
                t0 = toff[g]
                DE = D_PAD + 1
                COLT = 8 * DE
                JF = FIN * DE
                JH = H * DE
                MJF = FIN * D_PAD
                O_MK = JF
                NB = -(-G // 3)
                TH = G * H
                THF = TH * FIN
                xg = xgs[g]
                # fp16 cast, one op per tile for pipelining
                xb = wp.tile([128, G_MAX * 8 * DE_MAX], F16, tag="xb")
                for t in range(G):
                    nc.scalar.activation(
                        out=xb[:, t * COLT : (t + 1) * COLT],
                        in_=xg[:, t * COLT : (t + 1) * COLT], func=ACT.Copy)

                # a_dst: ad[s,(t,h)] = sum_f xslot*Ad  (from fp32 xg)
                pd = wp.tile([128, G_MAX * H * FIN], F32, tag="pd")
                nc.vector.tensor_tensor(
                    out=_ap(pd[:], 0, [[H * FIN, G], [FIN, H], [1, FIN]]),
                    in0=_ap(xg[:], D_PAD, [[COLT, G], [0, H], [DE, FIN]]),
                    in1=_ap(a78[:], 28, [[0, G], [1, H], [4, FIN]]),
                    op=OP.mult)
                adt = wp.tile([128, G_MAX * H], F32, tag="adt")
                nc.vector.tensor_reduce(
                    out=_ap(adt[:], 0, [[H, G], [1, H]]),
                    in_=_ap(pd[:], 0, [[H * FIN, G], [FIN, H], [1, FIN]]),
                    axis=AX, op=OP.add)
                adb = wp.tile([128, G_MAX * H], F16, tag="adb")
                nc.scalar.activation(out=adb[:, 0 : G * H],
                                     in_=adt[:, 0 : G * H], func=ACT.Copy)
                # a_src products: pa[s,(t,h,f,j)] = xb * AsE  (fp16 2x)
                pa = wp.tile([128, G_MAX * H * JF_MAX], F16, tag="pa")
                for t in range(G):
                    eng = nc.gpsimd if (D_PAD <= 20 and t % 2 == 1) else nc.vector
                    eng.tensor_tensor(
                        out=_ap(pa[:], t * H * JF,
                                [[JF, H], [DE, FIN], [1, DE]]),
                        in0=_ap(xb[:], t * COLT,
                                [[0, H], [DE, FIN], [1, DE]]),
                        in1=_ap(AsE[:], 0,
                                [[JF_MAX, H], [DE_MAX, FIN], [1, DE]]),
                        op=OP.mult)
                # tree-reduce over f (7 = 3+3+1): sE[s,(t,h,j)]
                q3 = wp.tile([128, G_MAX * H * 3 * DE_MAX], F16, tag="q3")
                nc.vector.tensor_tensor(
                    out=_ap(q3[:], 0, [[3 * DE, TH], [DE, 3], [1, DE]]),
                    in0=_ap(pa[:], 0, [[JF, TH], [DE, 3], [1, DE]]),
                    in1=_ap(pa[:], 3 * DE, [[JF, TH], [DE, 3], [1, DE]]),
                    op=OP.add)
                r1 = wp.tile([128, G_MAX * H * DE_MAX], F16, tag="r1")
                nc.vector.tensor_tensor(
                    out=_ap(r1[:], 0, [[DE, TH], [1, DE]]),
                    in0=_ap(q3[:], 0, [[3 * DE, TH], [1, DE]]),
                    in1=_ap(q3[:], DE, [[3 * DE, TH], [1, DE]]),
                    op=OP.add)
                r2 = wp.tile([128, G_MAX * H * DE_MAX], F16, tag="r2")
                nc.vector.tensor_tensor(
                    out=_ap(r2[:], 0, [[DE, TH], [1, DE]]),
                    in0=_ap(r1[:], 0, [[DE, TH], [1, DE]]),
                    in1=_ap(q3[:], 2 * DE, [[3 * DE, TH], [1, DE]]),
                    op=OP.add)
                sE = wp.tile([128, G_MAX * H * DE_MAX], F16, tag="sE")
                nc.vector.tensor_tensor(
                    out=_ap(sE[:], 0, [[DE, TH], [1, DE]]),
                    in0=_ap(r2[:], 0, [[DE, TH], [1, DE]]),
                    in1=_ap(pa[:], 6 * DE, [[JF, TH], [1, DE]]),
                    op=OP.add)
                # + mask, + a_dst bcast, leaky (gpsimd), then exp
                sF = wp.tile([128, G_MAX * H * DE_MAX], F16, tag="sF")
                (nc.vector if D_PAD <= 20 else nc.gpsimd).tensor_tensor(
                    out=_ap(sF[:], 0, [[JH, G], [DE, H], [1, DE]]),
                    in0=_ap(sE[:], 0, [[JH, G], [DE, H], [1, DE]]),
                    in1=_ap(xb[:], O_MK, [[COLT, G], [0, H], [1, DE]]),
                    op=OP.add)
                sG = wp.tile([128, G_MAX * H * DE_MAX], F16, tag="sG")
                (nc.vector if D_PAD <= 20 else nc.gpsimd).tensor_tensor(
                    out=_ap(sG[:], 0, [[JH, G], [DE, H], [1, DE]]),
                    in0=_ap(sF[:], 0, [[JH, G], [DE, H], [1, DE]]),
                    in1=_ap(adb[:], 0, [[H, G], [1, H], [0, DE]]),
                    op=OP.add)
                ezl = wp.tile([128, G_MAX * H * DE_MAX], F16, tag="ezl")
                nc.vector.scalar_tensor_tensor(
                    out=ezl[:, 0 : TH * DE], in0=sG[:, 0 : TH * DE],
                    scalar=NEG, in1=sG[:, 0 : TH * DE],
                    op0=OP.mult, op1=OP.max)
                mx = wp.tile([128, G_MAX * H], F16, tag="mx")
                nc.vector.tensor_reduce(
                    out=_ap(mx[:], 0, [[H, G], [1, H]]),
                    in_=_ap(ezl[:], 0, [[JH, G], [DE, H], [1, DE]]),
                    axis=AX, op=OP.max)
                ezm = wp.tile([128, G_MAX * H * DE_MAX], F16, tag="ezm")
                nc.gpsimd.tensor_tensor(
                    out=_ap(ezm[:], 0, [[JH, G], [DE, H], [1, DE]]),
                    in0=_ap(ezl[:], 0, [[JH, G], [DE, H], [1, DE]]),
                    in1=_ap(mx[:], 0, [[H, G], [1, H], [0, DE]]),
                    op=OP.subtract)
                ez = wp.tile([128, G_MAX * H * DE_MAX], F16, tag="ez")
                nc.scalar.activation(out=ez[:, 0 : TH * DE],
                                     in_=ezm[:, 0 : TH * DE], func=ACT.Exp)

                # denominators (fp32)
                den = wp.tile([128, G_MAX * H], F32, tag="den")
                nc.vector.tensor_reduce(
                    out=_ap(den[:], 0, [[H, G], [1, H]]),
                    in_=_ap(ez[:], 0, [[JH, G], [DE, H], [1, DE]]),
                    axis=AX, op=OP.add)

                # messages: pm[s,(t,h,f,j)] = ez * xb  (fp16 2x, j<D_PAD)
                pm = wp.tile([128, G_MAX * H * FIN * D_MAX], F16, tag="pm")
                for t in range(G):
                    eng = nc.gpsimd if (D_PAD <= 20 and t % 2 == 0) else nc.vector
                    eng.tensor_tensor(
                        out=_ap(pm[:], t * H * MJF,
                                [[MJF, H], [D_PAD, FIN], [1, D_PAD]]),
                        in0=_ap(ez[:], t * JH,
                                [[DE, H], [0, FIN], [1, D_PAD]]),
                        in1=_ap(xb[:], t * COLT,
                                [[0, H], [DE, FIN], [1, D_PAD]]),
                        op=OP.mult)
                # generic pairwise tree-reduce over j; odd leftovers folded
                # in at the end; small levels accumulate in fp32
                cur, stride, ncur = pm, D_PAD, D_PAD
                leftovers = []
                lvl = 0
                while ncur > 1:
                    half = ncur // 2
                    if ncur % 2:
                        leftovers.append((cur, stride, ncur - 1))
                    dt = F16 if half >= 4 else F32
                    hm = halves_max[lvl] if lvl < len(halves_max) else half
                    nxt = wp.tile([128, G_MAX * H * FIN * max(hm, half)],
                                  dt, tag=f"jt{lvl}")
                    nc.vector.tensor_tensor(
                        out=_ap(nxt[:], 0, [[half, THF], [1, half]]),
                        in0=_ap(cur[:], 0, [[stride, THF], [1, half]]),
                        in1=_ap(cur[:], half, [[stride, THF], [1, half]]),
                        op=OP.add)
                    cur, stride, ncur = nxt, half, half
                    lvl += 1
                for li, (buf, st, off) in enumerate(leftovers):
                    nxt = wp.tile([128, G_MAX * H * FIN], F32, tag=f"jl{li}")
                    nc.vector.tensor_tensor(
                        out=_ap(nxt[:], 0, [[1, THF]]),
                        in0=_ap(cur[:], 0, [[stride, THF]]),
                        in1=_ap(buf[:], off, [[st, THF]]),
                        op=OP.add)
                    cur, stride = nxt, 1
                SD = cur

                # normalize: Sn = SD/(den+eps), ones col 28 per tile
                rd = wp.tile([128, G_MAX * H], F32, tag="rd")
                nc.scalar.activation(out=rd[:, 0 : G * H],
                                     in_=den[:, 0 : G * H], func=ACT.Copy,
                                     bias=1e-16)
                nc.vector.reciprocal(out=rd[:, 0 : G * H],
                                     in_=rd[:, 0 : G * H])
                Sn = wp.tile([128, G_MAX * 32], F16, tag="Sn")
                nc.gpsimd.memset(_ap(Sn[:], 28, [[32, G]]), 1.0)
                nc.vector.tensor_tensor(
                    out=_ap(Sn[:], 0, [[32, G], [FIN, H], [1, FIN]]),
                    in0=_ap(SD[:], 0, [[H * FIN, G], [FIN, H], [1, FIN]]),
                    in1=_ap(rd[:], 0, [[H, G], [1, H], [0, FIN]]),
                    op=OP.mult)

                # transpose 3 tiles per PE pass; quadrant-aligned SnT
                SnTs = []
                for b in range(NB):
                    w = min(96, G * 32 - b * 96)
                    ps_t = pp_t.tile([96, 128], F16, tag="pst")
                    nc.tensor.transpose(out=ps_t[0:w, :],
                                        in_=Sn[:, b * 96 : b * 96 + w],
                                        identity=id16)
                    SnT = wp.tile([96, 128], F16, tag=f"snt{b}")
                    nc.scalar.activation(out=SnT[0:w, :], in_=ps_t[0:w, :],
                                         func=ACT.Copy)
                    SnTs.append(SnT)

                return SnTs

            def phaseB(g, SnTs):
                G, D_PAD = G_LIST[g], D_LIST[g]
                t0 = toff[g]
                # per tile: [o | q | -mu] = SnT.T @ WbFull, then LN + logits
                vs = wp.tile([128, G_MAX], F32, tag="vs")
                nm = wp.tile([128, G_MAX], F32, tag="nm")
                u_sb = wp.tile([128, G_MAX * CLS], F32, tag="u")
                lg = wp.tile([128, G_MAX * CLS], F32, tag="lg")
                for t in range(G):
                    b, tb = divmod(t, 3)
                    base = tb * 32
                    ps_o = pp_o.tile([128, 136], F32, tag="pso")
                    nc.tensor.matmul(
                        out=ps_o[:],
                        lhsT=SnTs[b][base : base + 29, :],
                        rhs=WbF[base : base + 29, :],
                        start=True, stop=True)
                    nc.vector.tensor_scalar(
                        out=nm[:, t : t + 1], in0=ps_o[:, 135:136],
                        scalar1=1.0, scalar2=None, op0=OP.mult)
                    sqt = wp.tile([128, HC], F32, tag="sqt")
                    nc.scalar.activation(
                        out=sqt[:], in_=ps_o[:, 0:HC], func=ACT.Square,
                        bias=nm[:, t : t + 1], accum_out=vs[:, t : t + 1])
                    nc.vector.scalar_tensor_tensor(
                        out=u_sb[:, t * CLS : (t + 1) * CLS],
                        in0=LC[:, CLS:14], scalar=ps_o[:, 135:136],
                        in1=ps_o[:, HC : HC + CLS],
                        op0=OP.mult, op1=OP.add)
                nc.scalar.activation(out=vs[:, 0:G], in_=vs[:, 0:G],
                                     func=ACT.Ln, scale=1.0 / HC,
                                     bias=CP[:, C_EPS : C_EPS + 1])
                nc.scalar.activation(out=vs[:, 0:G], in_=vs[:, 0:G],
                                     func=ACT.Exp, scale=-0.5)
                nc.vector.tensor_tensor(
                    out=_ap(lg[:], 0, [[CLS, G], [1, CLS]]),
                    in0=_ap(u_sb[:], 0, [[CLS, G], [1, CLS]]),
                    in1=_ap(vs[:], 0, [[1, G], [0, CLS]]), op=OP.mult)
                nc.vector.tensor_tensor(
                    out=_ap(lg[:], 0, [[CLS, G], [1, CLS]]),
                    in0=_ap(lg[:], 0, [[CLS, G], [1, CLS]]),
                    in1=_ap(LC[:], 0, [[0, G], [1, CLS]]), op=OP.add)
                nc.scalar.activation(out=lg[:, 0 : G * CLS],
                                     in_=lg[:, 0 : G * CLS], func=ACT.Exp)
                se = wp.tile([128, G_MAX], F32, tag="se")
                nc.vector.tensor_reduce(
                    out=se[:, 0:G],
                    in_=_ap(lg[:], 0, [[CLS, G], [1, CLS]]),
                    axis=AX, op=OP.add)
                nc.vector.reciprocal(out=se[:, 0:G], in_=se[:, 0:G])
                po = wp.tile([128, G_MAX * CLS], F32, tag="po")
                nc.gpsimd.tensor_tensor(
                    out=_ap(po[:], 0, [[CLS, G], [1, CLS]]),
                    in0=_ap(lg[:], 0, [[CLS, G], [1, CLS]]),
                    in1=_ap(se[:], 0, [[1, G], [0, CLS]]), op=OP.mult)
                nc.sync.dma_start(
                    out=d_out[:, t0 * CLS : (t0 + G) * CLS],
                    in_=po[:, 0 : G * CLS])


            # software pipeline: A0 A1 B0 A2 B1 ... B_last — each group's
            # psum-dependent tail sits after the next group's front half in
            # every engine queue, so engines never stall at group seams
            snts = {0: phaseA(0)}
            if NG > 1:
                snts[1] = phaseA(1)
            phaseB(0, snts[0])
            for g in range(2, NG):
                snts[g] = phaseA(g)
                phaseB(g - 1, snts[g - 1])
            if NG > 1:
                phaseB(NG - 1, snts[NG - 1])
    nc.compile()
    return nc


_CACHE = {}


def _program(T_pc, G_LIST, D_LIST):
    key = (T_pc, G_LIST, D_LIST)
    if key not in _CACHE:
        _CACHE[key] = _build(T_pc, G_LIST, D_LIST)
    return _CACHE[key]


# ---------------------------------------------------------------- entry
def kernel(x, edge_weight, W, att_src, att_dst, gat_bias, ln_w, ln_b,
           lin_W, lin_b, edge_index, ids):
    prep = _preprocess(np.asarray(x), np.asarray(edge_index),
                       np.asarray(ids))
    T_pc = prep["T_pc"]
    nc = _program(T_pc, prep["G_LIST"], prep["D_LIST"])
    cpack = _const_pack(W, att_src, att_dst, gat_bias, ln_w, ln_b,
                        lin_W, lin_b)

    in_maps = [{"xg2": prep["xg2"][c], "cpack": cpack}
               for c in range(NCORES)]

    if os.environ.get("KERNEL_SIM"):
        from concourse.bass_interp import CoreSim

        outs = []
        ncores = int(os.environ.get("KERNEL_SIM_CORES", "1"))
        for c in range(ncores):
            sim = CoreSim(nc, require_finite=False, require_nnan=False)
            for k, v in in_maps[c].items():
                sim.tensor(k)[:] = v
            sim.simulate()
            outs.append(sim.tensor("probs").copy())
        arr = np.stack(outs + [np.zeros_like(outs[0])] * (NCORES - ncores))
    else:
        trace = bool(int(os.environ.get("KERNEL_TRACE", "0")))
        res = bass_utils.run_bass_kernel_spmd(
            nc, in_maps, core_ids=list(range(NCORES)), trace=trace)
        if trace and res.exec_time_ns is not None:
            print(f"HW exec time: {res.exec_time_ns} ns")
        arr = np.stack([res.results[c]["probs"] for c in range(NCORES)])

    full = (arr.reshape(NCORES, 128, T_pc, CLS)
            .transpose(0, 2, 1, 3)
            .reshape(NCORES * T_pc * 128, CLS))
    return np.ascontiguousarray(
        full[prep["row_of_u"]][prep["inv"]], np.float32)


# revision 43
# speedup vs baseline: 1.1837x; 1.1158x over previous
"""GAT node-classification kernel for Trainium2 (8 NeuronCores, SPMD).

Strategy (dst-node graph partitioning per the sharding hint):
  - Only destination nodes appearing in `ids` affect the output; edges into
    other nodes are dead code and are dropped.
  - Destination nodes are sorted by in-degree and packed 128 per tile in
    rank order, dealt so all 8 cores share the same per-tile degree
    profile.  Tiles are batched into pipeline groups whose edge-slot width
    is the group's max degree (+1 pseudo-neighbor column carrying the
    node's own features for a_dst) — the attention/message work scales
    with the group width instead of the global max degree.
  - Per-edge attention logits are 7-term feature contractions on the
    Vector/GpSimd engines in fp16 (2-byte DVE fast modes); reductions are
    pairwise trees of tensor_tensor adds whose last levels accumulate in
    fp32.  A per-(tile,head) max is subtracted before exp (exact softmax
    shift) so fp16 exp inputs stay in range.  Weighted message sums stay
    in the rank-7 feature basis (sum(alpha*(x@W)) == (sum(alpha*x))@W).
  - Per tile the normalized sums (plus an appended ones column carrying
    the GAT bias and folded classifier bias) are transposed once on the
    PE (3 tiles per pass, quadrant-aligned) and hit a single fp16 matmul
    with rhs = [Wb;gb | Wb@linWp' | -rowsum/HC], yielding the GAT output
    o, the classifier projection q, and -mean(o) in one pass.  LayerNorm's
    affine transform is folded into the classifier weights; 1/std is
    applied after the matmul; rstd = exp(-0.5*ln(var)) so every
    activation shares one hardware function table.
  - All device inputs are packed host-side so each SBUF partition's data
    is contiguous in DRAM (one descriptor set per transfer); input DMAs
    are issued up-front on the in-order Sync queue; As/Ad are computed
    redundantly in every partition to avoid a DRAM broadcast roundtrip.

The host does no floating-point arithmetic on tensor values: it only
filters/sorts/permutes (sharding layout, gathers, block-diagonal placement
of W) and builds 0/1, eye, ones and 0/-60000 masks; all float math and all
dtype conversion runs on the NeuronCores.
"""
import os
import sys

sys.path.insert(0, "/opt/trn_rl_repo")

import numpy as np

import concourse.bass as bass
import concourse.bacc as bacc
import concourse.mybir as mybir
import concourse.tile as tile
from concourse import bass_utils
import concourse.bacc as _bacc_mod
import concourse.hw_specs as _hw_specs

_PIN_SET = "natural_log_exp_and_others"
_orig_get_tables = _hw_specs.get_activation_tables


def _pinned_tables(arch):
    """Route every activation to one table set (exp/ln/square/copy all
    coexist there) so the kernel pays a single ACT_TABLE_LOAD."""
    tabs = _orig_get_tables(arch)
    if _PIN_SET in tabs:
        tabs = {k: (v if k == _PIN_SET else set()) for k, v in tabs.items()}
    return tabs


_bacc_mod.get_activation_tables = _pinned_tables

N = 100000
FIN = 7
H = 4
C = 32
HC = H * C  # 128
CLS = 7
NEG = 0.2
NCORES = 8
F32 = mybir.dt.float32
F32R = mybir.dt.float32r
F16 = mybir.dt.float16

# const-pack column offsets (cursor-built)
_cur = 0


def _adv(w):
    global _cur
    o = _cur
    _cur += w
    return o


C_ID = _adv(128)     # identity [128,128]
C_WB = _adv(136)     # [Wb;gb] quadrant-replicated (128) | Wbc (7) | -rowsum/HC
C_WT = _adv(128)     # WT4G [128,125] (pad 3)
C_LIN = _adv(14)     # linW | (linWp written by device)
C_LNB = _adv(1)      # lnb column
C_ONE = _adv(1)      # ones column (adjacent: lhsT [128,2])
C_LNW = _adv(1)      # lnw column
C_EPS = _adv(1)      # 1e-5 column
C_E16 = _adv(1)      # 1e-16 column
C_SEL = _adv(14)     # rows 0:2 = keep-mask for [lbp | colsum] assembly
C_LB2 = _adv(14)     # row 0 cols 0:7 = lin_b, else 0
C_O2 = _adv(128)     # rows 0:2 = ones
NC = _cur + (-_cur % 8)


# ---------------------------------------------------------------- host prep
def _preprocess(x, edge_index, ids):
    src = np.asarray(edge_index[0], dtype=np.int64)
    dst = np.asarray(edge_index[1], dtype=np.int64)
    ids = np.asarray(ids, dtype=np.int64)
    x = np.asarray(x, np.float32)

    uids, inv = np.unique(ids, return_inverse=True)
    U = uids.shape[0]
    mark = np.full(N, -1, np.int64)
    mark[uids] = np.arange(U)

    dstc = mark[dst]
    keep = dstc >= 0
    es = src[keep]
    ed = dstc[keep]
    order = np.argsort(ed, kind="stable")
    es = es[order]
    ed = ed[order]
    cnt = np.bincount(ed, minlength=U).astype(np.int64)
    starts = np.zeros(U + 1, np.int64)
    np.cumsum(cnt, out=starts[1:])

    T_need = -(-U // 128)
    T_pc = -(-T_need // NCORES)
    T_pc += T_pc % 2
    T_tot = T_pc * NCORES
    Upad = T_tot * 128
    pads = Upad - U

    # degree-ascending rank: pad slots first, then nodes sorted by degree.
    # rank r -> tile t=r//1024, core c=(r//128)%8, slot s=r%128, so every
    # core sees the same per-tile degree profile.
    perm = np.argsort(cnt, kind="stable")
    rank_cnt = np.zeros(Upad, np.int64)
    rank_cnt[pads:] = cnt[perm]
    rank_uid = np.zeros(Upad, np.int64)
    rank_uid[pads:] = uids[perm]
    rank_uidx = np.full(Upad, -1, np.int64)
    rank_uidx[pads:] = perm
    rank_starts = np.zeros(Upad, np.int64)
    rank_starts[pads:] = starts[perm]

    # per-tile max degree -> groups of tiles with shared width
    tile_max = np.maximum(rank_cnt.reshape(T_pc, 8 * 128).max(1), 1)
    if T_pc == 10:
        G_LIST = [5, 4, 1]
    else:
        G_LIST = [4] * (T_pc // 4) + ([T_pc % 4] if T_pc % 4 else [])
    D_LIST = []
    t0 = 0
    for g in G_LIST:
        D_LIST.append(int(tile_max[t0 : t0 + g].max()))
        t0 += g

    blocks = []
    t0 = 0
    for G, D_PAD in zip(G_LIST, D_LIST):
        DE = D_PAD + 1
        COLT = 8 * DE
        r0, r1 = t0 * 1024, (t0 + G) * 1024
        nrow = r1 - r0
        sp = np.zeros((nrow, DE), np.int64)
        c0 = rank_cnt[r0:r1]
        st = rank_starts[r0:r1]
        # scatter edges: rows sorted by rank; edge k of row i at col k
        rows = np.repeat(np.arange(nrow), c0)
        cols = np.arange(rows.shape[0]) - np.repeat(
            np.cumsum(c0) - c0, c0)
        eidx = (np.repeat(st, c0) + cols)
        sp[rows, cols] = es[eidx]
        sp[:, D_PAD] = rank_uid[r0:r1]
        xgB = np.zeros((nrow, COLT), np.float32)
        xgB[:, : FIN * DE] = (
            x[sp.reshape(-1)].reshape(nrow, DE, FIN)
            .transpose(0, 2, 1).reshape(nrow, FIN * DE))
        j = np.arange(DE)[None, :]
        xgB[:, FIN * DE :] = np.where(
            j < c0[:, None], 0.0, -60000.0).astype(np.float32)
        blocks.append(np.ascontiguousarray(
            xgB.reshape(G, NCORES, 128, COLT)
            .transpose(1, 2, 0, 3)
            .reshape(NCORES, 128, G * COLT)))
        t0 += G
    xg2 = np.concatenate(blocks, axis=2)

    # output row of node-uidx u: device rows ordered (core, tile, slot)
    r = np.arange(pads, Upad)
    t, c, s = r // 1024, (r // 128) % 8, r % 128
    row_of_u = np.empty(U, np.int64)
    row_of_u[perm] = (c * T_pc + t) * 128 + s
    core_of_u = np.empty(U, np.int64)
    core_of_u[perm] = c

    return {"T_pc": T_pc, "G_LIST": tuple(G_LIST), "D_LIST": tuple(D_LIST),
            "xg2": xg2, "inv": inv, "row_of_u": row_of_u,
            "core_of_u": core_of_u}


def _const_pack(W, att_src, att_dst, gat_bias, ln_w, ln_b, lin_W, lin_b):
    W = np.ascontiguousarray(W, np.float32).reshape(FIN, HC)
    attS = np.ascontiguousarray(att_src, np.float32).reshape(HC)
    attD = np.ascontiguousarray(att_dst, np.float32).reshape(HC)
    gb = np.ascontiguousarray(gat_bias, np.float32).reshape(HC)
    cp = np.zeros((128, NC), np.float32)
    cp[:, C_ID : C_ID + 128] = np.eye(128, dtype=np.float32)
    # WbFull rows 32q+r: r<28 -> Wb row r ((h,f)=divmod(r,7)); r==28 -> gb
    wb = np.zeros((32, 128), np.float32)
    for r in range(28):
        h, f = divmod(r, FIN)
        wb[r, h * C : (h + 1) * C] = W[f, h * C : (h + 1) * C]
    wb[28, :] = gb
    for q in range(4):
        cp[32 * q : 32 * (q + 1), C_WB : C_WB + 128] = wb
    # WT4G col 32q+m: m<28 -> Wb row m transposed; m==28 -> gb
    wt = np.zeros((128, 32), np.float32)
    wt[:, :29] = wb[:29].T
    cp[:, C_WT : C_WT + 125] = np.tile(wt, (1, 4))[:, :125]
    cp[:, C_LIN : C_LIN + CLS] = np.ascontiguousarray(
        lin_W, np.float32).reshape(HC, CLS)
    cp[:, C_LNB] = np.ascontiguousarray(ln_b, np.float32).reshape(HC)
    cp[:, C_ONE] = 1.0
    cp[:, C_LNW] = np.ascontiguousarray(ln_w, np.float32).reshape(HC)
    cp[:, C_EPS] = 1e-5
    cp[:, C_E16] = 1e-16
    cp[0, C_SEL : C_SEL + CLS] = 1.0
    cp[1, C_SEL + CLS : C_SEL + 14] = 1.0
    cp[0, C_LB2 : C_LB2 + CLS] = np.ascontiguousarray(
        lin_b, np.float32).reshape(CLS)
    cp[0:2, C_O2 : C_O2 + 128] = 1.0
    cp2 = np.zeros((128, 284), np.float32)
    cp2[:, 0:128] = attS[:, None]        # lhsT: att value per hc, all cols
    cp2[:, 128:256] = attD[:, None]
    for f in range(FIN):                 # rhs: W placed with h-block mask
        for h in range(H):
            cp2[h * C : (h + 1) * C, 256 + f * H + h] = W[f, h * C : (h + 1) * C]
    return np.ascontiguousarray(np.concatenate([cp, cp2], axis=1))


def _ap(base, off_elems, dims):
    """AP with explicit free dims; dims = [[step, count], ...]."""
    return bass.AP(base.tensor, base.offset + off_elems,
                   [list(base.ap[0])] + dims)


# ---------------------------------------------------------------- program
def _build(T_pc, G_LIST, D_LIST):
    nc = bacc.Bacc("TRN2", target_bir_lowering=False, debug=False,
                   num_devices=NCORES)
    NG = len(G_LIST)
    G_MAX = max(G_LIST)
    D_MAX = max(D_LIST)
    DE_MAX = D_MAX + 1
    JF_MAX = FIN * DE_MAX
    COLT_L = [8 * (d + 1) for d in D_LIST]
    TOTC = sum(g * c for g, c in zip(G_LIST, COLT_L))
    # per-level max widths of the generic j-tree (for tile allocation)
    halves_max = []
    n = D_MAX
    while n > 1:
        halves_max.append(n // 2)
        n //= 2

    d_xg = nc.dram_tensor("xg2", [128, TOTC], F32, kind="ExternalInput")
    d_cp = nc.dram_tensor("cpack", [128, NC + 284], F32,
                          kind="ExternalInput")
    d_out = nc.dram_tensor("probs", [128, T_pc * CLS], F32,
                           kind="ExternalOutput")

    AX = mybir.AxisListType.X
    OP = mybir.AluOpType
    ACT = mybir.ActivationFunctionType

    with tile.TileContext(nc) as tc:
        with (
            tc.tile_pool(name="const", bufs=1) as cp,
            tc.tile_pool(name="work", bufs=2) as wp,
            tc.tile_pool(name="pp_p", bufs=1, space="PSUM") as pp_p,
            tc.tile_pool(name="pp_t", bufs=2, space="PSUM") as pp_t,
            tc.tile_pool(name="pp_o", bufs=2, space="PSUM") as pp_o,
        ):
            # ---- prologue: packed const DMAs, then on-device weight prep
            CP = cp.tile([128, NC + 284], F32, tag="CP")
            # att/W region lands first: it gates the As/Ad -> AsE chain
            nc.sync.dma_start(out=CP[:, NC : NC + 284],
                              in_=d_cp[:, NC : NC + 284])
            nc.sync.dma_start(out=CP[:, 0:NC], in_=d_cp[:, 0:NC])
            ident = CP[:, C_ID : C_ID + 128]

            # main-input DMAs issued up-front (in-order Sync queue)
            goff0 = [0]
            for g in range(NG):
                goff0.append(goff0[-1] + G_LIST[g] * COLT_L[g])
            xgs = {}
            for g in range(NG):
                w = G_LIST[g] * COLT_L[g]
                xgt = wp.tile([128, G_MAX * 8 * DE_MAX], F32, tag="xg")
                nc.sync.dma_start(out=xgt[:, 0:w],
                                  in_=d_xg[:, goff0[g] : goff0[g] + w])
                xgs[g] = xgt

            # As/Ad via PE broadcast matmuls (PE idle in the prologue):
            # out[p,(f,h)] = sum_hc att_w[hc] * Wplaced[hc,(f,h)], all p equal
            a78 = cp.tile([128, 56], F32, tag="a78")
            for w in range(2):
                ps_at = pp_p.tile([128, 28], F32, tag="psat")
                nc.tensor.matmul(
                    out=ps_at[:], lhsT=CP[:, NC + 128 * w : NC + 128 * (w + 1)],
                    rhs=CP[:, NC + 256 : NC + 284], start=True, stop=True)
                nc.scalar.copy(out=a78[:, 28 * w : 28 * (w + 1)], in_=ps_at[:])
            # expanded fp16 (h,f,j) table of As (a_dst handled separately;
            # the pseudo-neighbor column j=D_PAD is masked anyway)
            AsE = cp.tile([128, H * JF_MAX], F16, tag="AsE")
            nc.scalar.activation(
                out=_ap(AsE[:], 0, [[JF_MAX, H], [DE_MAX, FIN], [1, DE_MAX]]),
                in_=_ap(a78[:], 0, [[1, H], [4, FIN], [0, DE_MAX]]),
                func=ACT.Copy)

            # linWp = lnw * linW (cols 7:14 of lin2 region, inside CP)
            nc.vector.tensor_scalar(
                out=CP[:, C_LIN + CLS : C_LIN + 2 * CLS],
                in0=CP[:, C_LIN : C_LIN + CLS],
                scalar1=CP[:, C_LNW : C_LNW + 1], scalar2=None, op0=OP.mult)
            # Wbc[32q+m, k] = (Wb@linWp')[m,k] (m<28) / (gb@linWp')[k] (m=28)
            ps_w = pp_p.tile([125, CLS], F32, tag="psw")
            nc.tensor.matmul(
                out=ps_w[:], lhsT=CP[:, C_WT : C_WT + 125],
                rhs=CP[:, C_LIN + CLS : C_LIN + 2 * CLS],
                start=True, stop=True)
            WbF = cp.tile([128, 136], F16, tag="WbF")
            nc.scalar.activation(out=WbF[:, 0:128],
                                 in_=CP[:, C_WB : C_WB + 128], func=ACT.Copy)
            nc.scalar.activation(out=WbF[0:125, 128:135], in_=ps_w[:],
                                 func=ACT.Copy)
            id16 = cp.tile([128, 128], F16, tag="id16")
            nc.scalar.activation(out=id16[:], in_=ident, func=ACT.Copy)
            # wsum col: -(row sum of [Wb;gb]) / HC  -> matmul emits -mean(o)
            ws = cp.tile([128, 1], F32, tag="ws")
            nc.vector.tensor_reduce(
                out=ws[:], in_=CP[:, C_WB : C_WB + 128], axis=AX, op=OP.add)
            nc.scalar.activation(out=WbF[:, 135:136], in_=ws[:],
                                 func=ACT.Copy, scale=-1.0 / HC)
            # [lbp | colsum] broadcast rows
            ps_a = pp_p.tile([2, 14], F32, tag="psa")
            nc.tensor.matmul(
                out=ps_a[:], lhsT=CP[:, C_LNB : C_LNB + 2],
                rhs=CP[:, C_LIN : C_LIN + 14],
                start=True, stop=True)
            z2a = cp.tile([2, 14], F32, tag="z2a")
            nc.vector.tensor_tensor(out=z2a[:], in0=ps_a[:],
                                    in1=CP[0:2, C_SEL : C_SEL + 14],
                                    op=OP.mult)
            z2 = cp.tile([2, 14], F32, tag="z2")
            nc.vector.tensor_tensor(out=z2[:], in0=z2a[:],
                                    in1=CP[0:2, C_LB2 : C_LB2 + 14],
                                    op=OP.add)
            ps_b = pp_p.tile([128, 14], F32, tag="psb")
            nc.tensor.matmul(
                out=ps_b[:], lhsT=CP[0:2, C_O2 : C_O2 + 128],
                rhs=z2[:], start=True, stop=True)
            LC = cp.tile([128, 14], F32, tag="LC")
            nc.scalar.copy(out=LC[:], in_=ps_b[:])

            # ---- main loop: NG groups in a software pipeline
            toff = [0]
            for g in range(NG):
                toff.append(toff[-1] + G_LIST[g])

            def phaseA(g):
                G, D_PAD = G_LIST[g], D_LIST[g]
                t0 = toff[g]
                DE = D_PAD + 1
                COLT = 8 * DE
                JF = FIN * DE
                JH = H * DE
                MJF = FIN * D_PAD
                O_MK = JF
                NB = -(-G // 3)
                TH = G * H
                THF = TH * FIN
                xg = xgs[g]
                # fp16 cast, one op per tile for pipelining
                xb = wp.tile([128, G_MAX * 8 * DE_MAX], F16, tag="xb")
                for t in range(G):
                    nc.scalar.activation(
                        out=xb[:, t * COLT : (t + 1) * COLT],
                        in_=xg[:, t * COLT : (t + 1) * COLT], func=ACT.Copy)

                # a_dst: ad[s,(t,h)] = sum_f xslot*Ad  (from fp32 xg)
                pd = wp.tile([128, G_MAX * H * FIN], F32, tag="pd")
                nc.vector.tensor_tensor(
                    out=_ap(pd[:], 0, [[H * FIN, G], [FIN, H], [1, FIN]]),
                    in0=_ap(xg[:], D_PAD, [[COLT, G], [0, H], [DE, FIN]]),
                    in1=_ap(a78[:], 28, [[0, G], [1, H], [4, FIN]]),
                    op=OP.mult)
                adt = wp.tile([128, G_MAX * H], F32, tag="adt")
                nc.vector.tensor_reduce(
                    out=_ap(adt[:], 0, [[H, G], [1, H]]),
                    in_=_ap(pd[:], 0, [[H * FIN, G], [FIN, H], [1, FIN]]),
                    axis=AX, op=OP.add)
                adb = wp.tile([128, G_MAX * H], F16, tag="adb")
                nc.scalar.activation(out=adb[:, 0 : G * H],
                                     in_=adt[:, 0 : G * H], func=ACT.Copy)
                # a_src products: pa[s,(t,h,f,j)] = xb * AsE  (fp16 2x)
                pa = wp.tile([128, G_MAX * H * JF_MAX], F16, tag="pa")
                for t in range(G):
                    eng = nc.gpsimd if (D_PAD <= 20 and t % 2 == 1) else nc.vector
                    eng.tensor_tensor(
                        out=_ap(pa[:], t * H * JF,
                                [[JF, H], [DE, FIN], [1, DE]]),
                        in0=_ap(xb[:], t * COLT,
                                [[0, H], [DE, FIN], [1, DE]]),
                        in1=_ap(AsE[:], 0,
                                [[JF_MAX, H], [DE_MAX, FIN], [1, DE]]),
                        op=OP.mult)
                # tree-reduce over f (7 = 3+3+1): sE[s,(t,h,j)]
                q3 = wp.tile([128, G_MAX * H * 3 * DE_MAX], F16, tag="q3")
                nc.vector.tensor_tensor(
                    out=_ap(q3[:], 0, [[3 * DE, TH], [DE, 3], [1, DE]]),
                    in0=_ap(pa[:], 0, [[JF, TH], [DE, 3], [1, DE]]),
                    in1=_ap(pa[:], 3 * DE, [[JF, TH], [DE, 3], [1, DE]]),
                    op=OP.add)
                r1 = wp.tile([128, G_MAX * H * DE_MAX], F16, tag="r1")
                nc.vector.tensor_tensor(
                    out=_ap(r1[:], 0, [[DE, TH], [1, DE]]),
                    in0=_ap(q3[:], 0, [[3 * DE, TH], [1, DE]]),
                    in1=_ap(q3[:], DE, [[3 * DE, TH], [1, DE]]),
                    op=OP.add)
                r2 = wp.tile([128, G_MAX * H * DE_MAX], F16, tag="r2")
                nc.vector.tensor_tensor(
                    out=_ap(r2[:], 0, [[DE, TH], [1, DE]]),
                    in0=_ap(r1[:], 0, [[DE, TH], [1, DE]]),
                    in1=_ap(q3[:], 2 * DE, [[3 * DE, TH], [1, DE]]),
                    op=OP.add)
                sE = wp.tile([128, G_MAX * H * DE_MAX], F16, tag="sE")
                nc.vector.tensor_tensor(
                    out=_ap(sE[:], 0, [[DE, TH], [1, DE]]),
                    in0=_ap(r2[:], 0, [[DE, TH], [1, DE]]),
                    in1=_ap(pa[:], 6 * DE, [[JF, TH], [1, DE]]),
                    op=OP.add)
                # + mask, + a_dst bcast, leaky (gpsimd), then exp
                sF = wp.tile([128, G_MAX * H * DE_MAX], F16, tag="sF")
                (nc.vector if D_PAD <= 20 else nc.gpsimd).tensor_tensor(
                    out=_ap(sF[:], 0, [[JH, G], [DE, H], [1, DE]]),
                    in0=_ap(sE[:], 0, [[JH, G], [DE, H], [1, DE]]),
                    in1=_ap(xb[:], O_MK, [[COLT, G], [0, H], [1, DE]]),
                    op=OP.add)
                sG = wp.tile([128, G_MAX * H * DE_MAX], F16, tag="sG")
                (nc.vector if D_PAD <= 20 else nc.gpsimd).tensor_tensor(
                    out=_ap(sG[:], 0, [[JH, G], [DE, H], [1, DE]]),
                    in0=_ap(sF[:], 0, [[JH, G], [DE, H], [1, DE]]),
                    in1=_ap(adb[:], 0, [[H, G], [1, H], [0, DE]]),
                    op=OP.add)
                ezl = wp.tile([128, G_MAX * H * DE_MAX], F16, tag="ezl")
                nc.vector.scalar_tensor_tensor(
                    out=ezl[:, 0 : TH * DE], in0=sG[:, 0 : TH * DE],
                    scalar=NEG, in1=sG[:, 0 : TH * DE],
                    op0=OP.mult, op1=OP.max)
                mx = wp.tile([128, G_MAX * H], F16, tag="mx")
                nc.vector.tensor_reduce(
                    out=_ap(mx[:], 0, [[H, G], [1, H]]),
                    in_=_ap(ezl[:], 0, [[JH, G], [DE, H], [1, DE]]),
                    axis=AX, op=OP.max)
                ezm = wp.tile([128, G_MAX * H * DE_MAX], F16, tag="ezm")
                nc.gpsimd.tensor_tensor(
                    out=_ap(ezm[:], 0, [[JH, G], [DE, H], [1, DE]]),
                    in0=_ap(ezl[:], 0, [[JH, G], [DE, H], [1, DE]]),
                    in1=_ap(mx[:], 0, [[H, G], [1, H], [0, DE]]),
                    op=OP.subtract)
                ez = wp.tile([128, G_MAX * H * DE_MAX], F16, tag="ez")
                nc.scalar.activation(out=ez[:, 0 : TH * DE],
                                     in_=ezm[:, 0 : TH * DE], func=ACT.Exp)

                # denominators (fp32)
                den = wp.tile([128, G_MAX * H], F32, tag="den")
                nc.vector.tensor_reduce(
                    out=_ap(den[:], 0, [[H, G], [1, H]]),
                    in_=_ap(ez[:], 0, [[JH, G], [DE, H], [1, DE]]),
                    axis=AX, op=OP.add)

                # messages: pm[s,(t,h,f,j)] = ez * xb  (fp16 2x, j<D_PAD)
                pm = wp.tile([128, G_MAX * H * FIN * D_MAX], F16, tag="pm")
                for t in range(G):
                    eng = nc.gpsimd if (D_PAD <= 20 and t % 2 == 0) else nc.vector
                    eng.tensor_tensor(
                        out=_ap(pm[:], t * H * MJF,
                                [[MJF, H], [D_PAD, FIN], [1, D_PAD]]),
                        in0=_ap(ez[:], t * JH,
                                [[DE, H], [0, FIN], [1, D_PAD]]),
                        in1=_ap(xb[:], t * COLT,
                                [[0, H], [DE, FIN], [1, D_PAD]]),
                        op=OP.mult)
                # generic pairwise tree-reduce over j; odd leftovers folded
                # in at the end; small levels accumulate in fp32
                cur, stride, ncur = pm, D_PAD, D_PAD
                leftovers = []
                lvl = 0
                while ncur > 1:
                    half = ncur // 2
                    if ncur % 2:
                        leftovers.append((cur, stride, ncur - 1))
                    dt = F16 if half >= 4 else F32
                    hm = halves_max[lvl] if lvl < len(halves_max) else half
                    nxt = wp.tile([128, G_MAX * H * FIN * max(hm, half)],
                                  dt, tag=f"jt{lvl}")
                    nc.vector.tensor_tensor(
                        out=_ap(nxt[:], 0, [[half, THF], [1, half]]),
                        in0=_ap(cur[:], 0, [[stride, THF], [1, half]]),
                        in1=_ap(cur[:], half, [[stride, THF], [1, half]]),
                        op=OP.add)
                    cur, stride, ncur = nxt, half, half
                    lvl += 1
                for li, (buf, st, off) in enumerate(leftovers):
                    nxt = wp.tile([128, G_MAX * H * FIN], F32, tag=f"jl{li}")
                    nc.vector.tensor_tensor(
                        out=_ap(nxt[:], 0, [[1, THF]]),
                        in0=_ap(cur[:], 0, [[stride, THF]]),
                        in1=_ap(buf[:], off, [[st, THF]]),
                        op=OP.add)
                    cur, stride = nxt, 1
                SD = cur

                # normalize: Sn = SD/(den+eps), ones col 28 per tile
                rd = wp.tile([128, G_MAX * H], F32, tag="rd")
                nc.scalar.activation(out=rd[:, 0 : G * H],
                                     in_=den[:, 0 : G * H], func=ACT.Copy,
                                     bias=1e-16)
                nc.vector.reciprocal(out=rd[:, 0 : G * H],
                                     in_=rd[:, 0 : G * H])
                Sn = wp.tile([128, G_MAX * 32], F16, tag="Sn")
                nc.gpsimd.memset(_ap(Sn[:], 28, [[32, G]]), 1.0)
                nc.vector.tensor_tensor(
                    out=_ap(Sn[:], 0, [[32, G], [FIN, H], [1, FIN]]),
                    in0=_ap(SD[:], 0, [[H * FIN, G], [FIN, H], [1, FIN]]),
                    in1=_ap(rd[:], 0, [[H, G], [1, H], [0, FIN]]),
                    op=OP.mult)

                # transpose 3 tiles per PE pass; quadrant-aligned SnT
                SnTs = []
                for b in range(NB):
                    w = min(96, G * 32 - b * 96)
                    ps_t = pp_t.tile([96, 128], F16, tag="pst")
                    nc.tensor.transpose(out=ps_t[0:w, :],
                                        in_=Sn[:, b * 96 : b * 96 + w],
                                        identity=id16)
                    SnT = wp.tile([96, 128], F16, tag=f"snt{b}")
                    nc.scalar.activation(out=SnT[0:w, :], in_=ps_t[0:w, :],
                                         func=ACT.Copy)
                    SnTs.append(SnT)

                return SnTs

            def phaseB(g, SnTs):
                G, D_PAD = G_LIST[g], D_LIST[g]
                t0 = toff[g]
                # per tile: [o | q | -mu] = SnT.T @ WbFull, then LN + logits
                vs = wp.tile([128, G_MAX], F32, tag="vs")
                nm = wp.tile([128, G_MAX], F32, tag="nm")
                u_sb = wp.tile([128, G_MAX * CLS], F32, tag="u")
                lg = wp.tile([128, G_MAX * CLS], F32, tag="lg")
                for t in range(G):
                    b, tb = divmod(t, 3)
                    base = tb * 32
                    ps_o = pp_o.tile([128, 136], F32, tag="pso")
                    nc.tensor.matmul(
                        out=ps_o[:],
                        lhsT=SnTs[b][base : base + 29, :],
                        rhs=WbF[base : base + 29, :],
                        start=True, stop=True)
                    nc.scalar.activation(
                        out=nm[:, t : t + 1], in_=ps_o[:, 135:136],
                        func=ACT.Copy)
                    sqt = wp.tile([128, HC], F32, tag="sqt")
                    nc.scalar.activation(
                        out=sqt[:], in_=ps_o[:, 0:HC], func=ACT.Square,
                        bias=nm[:, t : t + 1], accum_out=vs[:, t : t + 1])
                    nc.vector.scalar_tensor_tensor(
                        out=u_sb[:, t * CLS : (t + 1) * CLS],
                        in0=LC[:, CLS:14], scalar=ps_o[:, 135:136],
                        in1=ps_o[:, HC : HC + CLS],
                        op0=OP.mult, op1=OP.add)
                nc.scalar.activation(out=vs[:, 0:G], in_=vs[:, 0:G],
                                     func=ACT.Ln, scale=1.0 / HC,
                                     bias=CP[:, C_EPS : C_EPS + 1])
                nc.scalar.activation(out=vs[:, 0:G], in_=vs[:, 0:G],
                                     func=ACT.Exp, scale=-0.5)
                nc.vector.tensor_tensor(
                    out=_ap(lg[:], 0, [[CLS, G], [1, CLS]]),
                    in0=_ap(u_sb[:], 0, [[CLS, G], [1, CLS]]),
                    in1=_ap(vs[:], 0, [[1, G], [0, CLS]]), op=OP.mult)
                nc.vector.tensor_tensor(
                    out=_ap(lg[:], 0, [[CLS, G], [1, CLS]]),
                    in0=_ap(lg[:], 0, [[CLS, G], [1, CLS]]),
                    in1=_ap(LC[:], 0, [[0, G], [1, CLS]]), op=OP.add)
                nc.scalar.activation(out=lg[:, 0 : G * CLS],
                                     in_=lg[:, 0 : G * CLS], func=ACT.Exp)
                se = wp.tile([128, G_MAX], F32, tag="se")
                nc.vector.tensor_reduce(
                    out=se[:, 0:G],
                    in_=_ap(lg[:], 0, [[CLS, G], [1, CLS]]),
                    axis=AX, op=OP.add)
                nc.vector.reciprocal(out=se[:, 0:G], in_=se[:, 0:G])
                po = wp.tile([128, G_MAX * CLS], F32, tag="po")
                nc.gpsimd.tensor_tensor(
                    out=_ap(po[:], 0, [[CLS, G], [1, CLS]]),
                    in0=_ap(lg[:], 0, [[CLS, G], [1, CLS]]),
                    in1=_ap(se[:], 0, [[1, G], [0, CLS]]), op=OP.mult)
                nc.sync.dma_start(
                    out=d_out[:, t0 * CLS : (t0 + G) * CLS],
                    in_=po[:, 0 : G * CLS])


            # software pipeline: A0 A1 B0 A2 B1 ... B_last — each group's
            # psum-dependent tail sits after the next group's front half in
            # every engine queue, so engines never stall at group seams
            snts = {0: phaseA(0)}
            if NG > 1:
                snts[1] = phaseA(1)
            phaseB(0, snts[0])
            for g in range(2, NG):
                snts[g] = phaseA(g)
                phaseB(g - 1, snts[g - 1])
            if NG > 1:
                phaseB(NG - 1, snts[NG - 1])
    nc.compile()
    return nc


_CACHE = {}


def _program(T_pc, G_LIST, D_LIST):
    key = (T_pc, G_LIST, D_LIST)
    if key not in _CACHE:
        _CACHE[key] = _build(T_pc, G_LIST, D_LIST)
    return _CACHE[key]


# ---------------------------------------------------------------- entry
def kernel(x, edge_weight, W, att_src, att_dst, gat_bias, ln_w, ln_b,
           lin_W, lin_b, edge_index, ids):
    prep = _preprocess(np.asarray(x), np.asarray(edge_index),
                       np.asarray(ids))
    T_pc = prep["T_pc"]
    nc = _program(T_pc, prep["G_LIST"], prep["D_LIST"])
    cpack = _const_pack(W, att_src, att_dst, gat_bias, ln_w, ln_b,
                        lin_W, lin_b)

    in_maps = [{"xg2": prep["xg2"][c], "cpack": cpack}
               for c in range(NCORES)]

    if os.environ.get("KERNEL_SIM"):
        from concourse.bass_interp import CoreSim

        outs = []
        ncores = int(os.environ.get("KERNEL_SIM_CORES", "1"))
        for c in range(ncores):
            sim = CoreSim(nc, require_finite=False, require_nnan=False)
            for k, v in in_maps[c].items():
                sim.tensor(k)[:] = v
            sim.simulate()
            outs.append(sim.tensor("probs").copy())
        arr = np.stack(outs + [np.zeros_like(outs[0])] * (NCORES - ncores))
    else:
        trace = bool(int(os.environ.get("KERNEL_TRACE", "0")))
        res = bass_utils.run_bass_kernel_spmd(
            nc, in_maps, core_ids=list(range(NCORES)), trace=trace)
        if trace and res.exec_time_ns is not None:
            print(f"HW exec time: {res.exec_time_ns} ns")
        arr = np.stack([res.results[c]["probs"] for c in range(NCORES)])

    full = (arr.reshape(NCORES, 128, T_pc, CLS)
            .transpose(0, 2, 1, 3)
            .reshape(NCORES * T_pc * 128, CLS))
    return np.ascontiguousarray(
        full[prep["row_of_u"]][prep["inv"]], np.float32)
